# revision 1
# baseline (speedup 1.0000x reference)
"""Trainium2 Bass kernel for nn_BasicCGInducer (CKY inside algorithm for a
categorial-grammar inducer).

Strategy (8 NeuronCores):
  - Data-parallel over sentences: core j handles sentences 4j..4j+3.
  - Emission log-partition (the big [C,V] softmax denominator) is
    tensor-parallel over vocab: each core computes sum_v exp(logits) for a
    4000-column V-shard, then one AllReduce of [C] partial sums.
  - Everything else (grammar tables, split-MLP, beta1, CKY) is computed
    per-core on its sentence shard in scaled-exp space (no logsumexp on the
    hot path; per-span running max scales).

kernel(**inputs) takes FULL inputs, shards on host, runs one SPMD bass
program on cores 0-7, and reassembles the [32] output.
"""
import sys
import contextlib

sys.path.insert(0, "/opt/trn_rl_repo")

import numpy as np

import concourse.bass as bass
import concourse.bacc as bacc
import concourse.mybir as mybir
import concourse.tile as tile
from concourse.ap import AP
from concourse import bass_utils

F32 = mybir.dt.float32
ALU = mybir.AluOpType
ACTF = mybir.ActivationFunctionType
AXIS = mybir.AxisListType

# ---------------------------------------------------------------- constants
P4 = 4          # primitive cats
NF = 36         # non-functor cats
C = 2596        # total cats
CP = 2688       # padded C (21 * 128)
NT = CP // 128  # 21 c-tiles
D = 64
B = 32          # total sentences
NCORES = 8
BLOC = B // NCORES  # 4 sentences per core
V = 32000
BLK = 80        # per-level block stride in chart tensors
NEGB = -1.0e5   # bias for padded vocab columns


class Cfg:
    def __init__(self, n=32, v_loc=4000, n_cores=8):
        self.n = n                      # sentence length
        self.v_loc = v_loc              # vocab shard per core
        self.v_pad = ((v_loc + 511) // 512) * 512
        self.n_cores = n_cores
        self.pairs = 4 * n              # (i, b) pairs on partitions


# ------------------------------------------------------------ functor maps
def lf_block_offsets(op):
    """c = off + {A: 4r+a | B: 32r+(a-4) | C: 36(r-4)+a} per derivation of
    the deterministic functor-id tables. op=0 -> l_functors, 1 -> r_functors."""
    return {
        "A": 4 + 16 * op,            # res<4, arg<4 : c = A + 4*res + arg
        "B": 36 + 1280 * op,         # res<4, arg>=4: c = B + 32*res + (arg-4)
        "C": 164 + 1280 * op,        # res>=4      : c = C0 + 36*(res-4) + arg
    }


def check_functor_tables(l_functors, r_functors):
    for op, tab in ((0, l_functors), (1, r_functors)):
        off = lf_block_offsets(op)
        exp = np.zeros((NF, NF), np.int64)  # [arg, res]
        for res in range(NF):
            for arg in range(NF):
                if res < P4 and arg < P4:
                    exp[arg, res] = off["A"] + 4 * res + arg
                elif res < P4:
                    exp[arg, res] = off["B"] + 32 * res + (arg - 4)
                else:
                    exp[arg, res] = off["C"] + 36 * (res - 4) + arg
        assert np.array_equal(np.asarray(tab, np.int64), exp), (
            f"functor table structure mismatch (op={op})")


# ---------------------------------------------------------------- AP helper
def mk(t, parts, off, dims, base_part=0):
    """Raw AP on tile t: partition range [base_part, base_part+parts),
    free offset `off` (elements), extra free dims [[step, count], ...]."""
    w = t.ap[0][0]
    return AP(t.tensor, t.offset + base_part * w + off, [[w, parts]] + dims)


# ============================================================ device program
def build_program(cfg: Cfg):
    nc = bacc.Bacc("TRN2", target_bir_lowering=False, debug=False,
                   num_devices=cfg.n_cores)
    d = {
        "ntembT": nc.dram_tensor("ntembT", [65, CP], F32,
                                 kind="ExternalInput"),
        "vocabW": nc.dram_tensor("vocabW", [65, cfg.v_pad], F32,
                                 kind="ExternalInput"),
        "wordW": nc.dram_tensor("wordW", [66, cfg.pairs], F32,
                                kind="ExternalInput"),
        "mlpW": nc.dram_tensor("mlpW", [64, 322], F32, kind="ExternalInput"),
        "mlpB": nc.dram_tensor("mlpB", [64, 8], F32, kind="ExternalInput"),
        "ruleWb": nc.dram_tensor("ruleWb", [36, 144], F32,
                                 kind="ExternalInput"),
        "smallv": nc.dram_tensor("smallv", [1, 16], F32,
                                 kind="ExternalInput"),
        "out": nc.dram_tensor("out_nll", [BLOC, 1], F32,
                              kind="ExternalOutput"),
    }
    with tile.TileContext(nc) as tc:
        _trace(tc, cfg, d)
    nc.compile()
    return nc


def _trace(tc, cfg, d):
    nc = tc.nc
    n, PAIRS, VP = cfg.n, cfg.pairs, cfg.v_pad
    NV = VP // 512                    # 512-col v-tiles per core
    NHALF = (NV + 3) // 4             # ACT chunks of up to 4 v-tiles
    HW = CP // 2                      # MLP half width (1344)

    es = contextlib.ExitStack()
    keep = es.enter_context(tc.tile_pool(name="keep", bufs=1))
    dram = es.enter_context(tc.tile_pool(name="dram", bufs=1, space="DRAM"))

    # ---------------- long-lived tensors
    chartA = keep.tile([PAIRS, (n + 1) * BLK], F32)
    chartE = keep.tile([PAIRS, (n + 1) * BLK], F32)
    WA = keep.tile([PAIRS, 1312], F32)
    WB = keep.tile([PAIRS, 1312], F32)
    glR = keep.tile([128, 1296], F32)
    grR = keep.tile([128, 1296], F32)
    M1 = keep.tile([PAIRS, 2], F32)
    mlpB = keep.tile([64, 8], F32)
    smallv = keep.tile([1, 16], F32)
    sumexp_parts = keep.tile([128, NT * NHALF], F32)
    sumexp_loc = keep.tile([128, NT], F32)
    sumexp_g = keep.tile([128, NT], F32)
    lse21 = keep.tile([128, NT], F32)
    s0E = keep.tile([1, NF], F32)
    db = keep.tile([1, 2], F32)
    rsRep = keep.tile([4, 4], F32)
    fin = keep.tile([4, 8], F32)

    nc.sync.dma_start(mlpB[:], d["mlpB"][:])
    nc.sync.dma_start(smallv[:], d["smallv"][:])
    nc.gpsimd.memset(chartA[:], 0.0)
    nc.gpsimd.memset(chartE[:], 0.0)

    ph1 = contextlib.ExitStack()
    p1 = ph1.enter_context(tc.tile_pool(name="ph1", bufs=1))
    ntembT = p1.tile([65, CP], F32)
    vocabW = p1.tile([65, VP], F32)
    wordW = p1.tile([66, PAIRS], F32)
    mlpW = p1.tile([64, 322], F32)
    ruleWb = p1.tile([36, 144], F32)
    lse_row = p1.tile([1, CP], F32)
    adj = p1.tile([1, CP], F32)
    rhs_b = p1.tile([66, CP], F32)
    beta1E = p1.tile([PAIRS, CP], F32)
    ruleflat = p1.tile([1, 36 * 72], F32)

    nc.sync.dma_start(ntembT[:], d["ntembT"][:])
    nc.sync.dma_start(vocabW[:], d["vocabW"][:])
    nc.sync.dma_start(wordW[:], d["wordW"][:])
    nc.sync.dma_start(mlpW[:], d["mlpW"][:])
    nc.sync.dma_start(ruleWb[:], d["ruleWb"][:])

    # =======================================================================
    # Phase 1: emission partition function (exp in place in PSUM + accum_out)
    # =======================================================================
    with tc.tile_pool(name="psum_e", bufs=2, space="PSUM") as pse, \
         tc.tile_pool(name="scr_e", bufs=2) as scre:
        for ct in range(NT):
            for h in range(NHALF):
                vt0 = h * 4
                nvt = min(4, NV - vt0)
                pt = pse.tile([128, 512 * nvt], F32, tag="pse")
                for vt in range(nvt):
                    nc.tensor.matmul(
                        pt[:, vt * 512:(vt + 1) * 512],
                        ntembT[:, ct * 128:(ct + 1) * 128],
                        vocabW[:, (vt0 + vt) * 512:(vt0 + vt + 1) * 512],
                        start=True, stop=True)
                sce = scre.tile([128, 512 * 4], F32, tag="scre")
                nc.scalar.activation(
                    sce[:, 0:512 * nvt], pt[:], ACTF.Exp,
                    accum_out=sumexp_parts[:, ct * NHALF + h:
                                           ct * NHALF + h + 1])

    if NHALF > 1:
        nc.vector.tensor_reduce(
            sumexp_loc[:],
            mk(sumexp_parts, 128, 0, [[NHALF, NT], [1, NHALF]]),
            axis=AXIS.X, op=ALU.add)
    else:
        nc.vector.tensor_copy(sumexp_loc[:], sumexp_parts[:, 0:NT])

    # AllReduce over cores via DRAM bounce
    cc_in = dram.tile([128, NT], F32)
    cc_out = dram.tile([128, NT], F32)
    nc.sync.dma_start(cc_in[:], sumexp_loc[:])
    nc.gpsimd.collective_compute(
        "AllReduce", ALU.add,
        replica_groups=[list(range(cfg.n_cores))],
        ins=[cc_in[:].opt()], outs=[cc_out[:].opt()])
    nc.sync.dma_start(sumexp_g[:], cc_out[:])

    nc.scalar.activation(lse21[:], sumexp_g[:], ACTF.Ln)
    # rearrange [128, NT] -> [1, CP]  (c = ct*128 + p) via DRAM bounce
    lse_d = dram.tile([128, NT], F32)
    nc.sync.dma_start(lse_d[:], lse21[:])
    nc.sync.dma_start(
        AP(lse_row.tensor, lse_row.offset,
           [[lse_row.ap[0][0], 1], [128, NT], [1, 128]]),
        AP(lse_d.tensor, lse_d.offset, [[lse_d.ap[0][0], 1], [1, NT], [NT, 128]]))

    # =======================================================================
    # Phase 2: split MLP (transposed layout hT [64, *]), rule tables, root
    # =======================================================================
    nc.vector.tensor_tensor(db[:, 0:1], smallv[:, 0:1], smallv[:, 1:2],
                            op=ALU.subtract)
    nc.vector.tensor_scalar_mul(db[:, 1:2], db[:, 0:1], -1.0)

    with tc.tile_pool(name="mlp", bufs=1) as mlp:
        hA = mlp.tile([64, HW], F32, tag="hA")
        hB = mlp.tile([64, HW], F32, tag="hB")
        hC = mlp.tile([64, HW], F32, tag="hC")
        s_rows = mlp.tile([2, HW], F32, tag="srows")
        w1 = mlp.tile([1, HW], F32, tag="w1")
        w2 = mlp.tile([1, HW], F32, tag="w2")
        w3 = mlp.tile([1, HW], F32, tag="w3")

        for half in range(2):
            base = half * HW

            def dense_relu(dst, col0, rhs, bias_col, res_add=None, rb=0,
                           func=ACTF.Relu):
                with tc.tile_pool(name="psum_m", bufs=2,
                                  space="PSUM") as psm:
                    for c0 in range(0, HW, 512):
                        c1 = min(c0 + 512, HW)
                        pm = psm.tile([64, 512], F32, tag="psm")
                        nc.tensor.matmul(pm[:, 0:c1 - c0],
                                         mlpW[:, col0:col0 + 64],
                                         rhs[0:64, rb + c0:rb + c1],
                                         start=True, stop=True)
                        nc.scalar.activation(
                            dst[:, c0:c1], pm[:, 0:c1 - c0], func,
                            bias=mlpB[:, bias_col:bias_col + 1])
                        if res_add is not None:
                            nc.vector.tensor_tensor(
                                dst[:, c0:c1], dst[:, c0:c1],
                                res_add[:, c0:c1], op=ALU.add)

            dense_relu(hA, 0, ntembT, 0, rb=base,
                       func=ACTF.Identity)           # h1 (linear)
            dense_relu(hB, 64, hA, 1)                   # t = relu(h1 W + b)
            dense_relu(hC, 128, hB, 2, res_add=hA)      # h2
            dense_relu(hB, 192, hC, 3)                  # t2
            dense_relu(hA, 256, hB, 4, res_add=hC)      # h3

            with tc.tile_pool(name="psum_s", bufs=2, space="PSUM") as pss:
                for c0 in range(0, HW, 512):
                    c1 = min(c0 + 512, HW)
                    ps = pss.tile([2, 512], F32, tag="pss")
                    nc.tensor.matmul(ps[:, 0:c1 - c0], mlpW[:, 320:322],
                                     hA[0:64, c0:c1], start=True, stop=True)
                    nc.vector.tensor_copy(s_rows[:, c0:c1], ps[:, 0:c1 - c0])

            # d = s0 - s1 (s1 via DMA to partition 0)
            nc.sync.dma_start(w1[:], s_rows[1:2, :])
            nc.vector.tensor_tensor(w2[:], s_rows[0:1, :], w1[:],
                                    op=ALU.subtract)
            # y = d + db;  softplus(y) = max(y,0) + ln(1+exp(-|y|))
            y = w2
            nc.vector.tensor_scalar_add(y[:], y[:], db[:, 0:1])
            nc.scalar.activation(w1[:], y[:], ACTF.Abs)
            nc.scalar.activation(w1[:], w1[:], ACTF.Exp, scale=-1.0)
            nc.scalar.activation(w1[:], w1[:], ACTF.Ln, bias=1.0)
            nc.vector.tensor_scalar_max(w3[:], y[:], 0.0)
            nc.vector.tensor_tensor(w3[:], w3[:], w1[:], op=ALU.add)  # sp
            # adj = -softplus(y) - lse
            nc.vector.scalar_tensor_tensor(
                adj[:, base:base + HW], w3[:], -1.0,
                lse_row[:, base:base + HW], op0=ALU.mult, op1=ALU.subtract)
            if half == 0:
                # split0 = -softplus(-y) = y - softplus(y); split0E = exp
                nc.vector.tensor_tensor(s0E[:], y[:, 0:NF], w3[:, 0:NF],
                                        op=ALU.subtract)
                nc.scalar.activation(s0E[:], s0E[:], ACTF.Exp)

    # rule tables: softmax over 72 per res row
    rsum = keep.tile([36, 72], F32)
    rmax = keep.tile([36, 2], F32)
    rsumexp = keep.tile([36, 2], F32)
    nc.vector.tensor_tensor(rsum[:], ruleWb[:, 0:72], ruleWb[:, 72:144],
                            op=ALU.add)
    nc.vector.tensor_reduce(rmax[:, 0:1], rsum[:], axis=AXIS.X, op=ALU.max)
    nc.vector.tensor_scalar_mul(rmax[:, 1:2], rmax[:, 0:1], -1.0)
    nc.scalar.activation(rsum[:], rsum[:], ACTF.Exp, bias=rmax[:, 1:2],
                         accum_out=rsumexp[:, 0:1])
    nc.vector.reciprocal(rsumexp[:, 1:2], rsumexp[:, 0:1])
    nc.vector.tensor_scalar_mul(rsum[:], rsum[:], rsumexp[:, 1:2])

    # flatten ruleEn to [1, 2592] via DRAM, then G-flats replicated
    rule_d = dram.tile([36, 72], F32)
    nc.sync.dma_start(rule_d[:], rsum[:])
    nc.sync.dma_start(
        AP(ruleflat.tensor, ruleflat.offset,
           [[ruleflat.ap[0][0], 1], [1, 36 * 72]]),
        rule_d[:])
    g_d = dram.tile([2, 1296], F32)
    gtmp = keep.tile([1, 1296], F32)
    for row, off in ((0, 0), (1, 36)):   # 0: Gl (larg), 1: Gr (rarg)
        nc.vector.tensor_tensor(
            gtmp[:],
            mk(ruleflat, 1, off, [[72, 36], [1, 36]]),
            mk(s0E, 1, 0, [[1, 36], [0, 36]]),
            op=ALU.mult)
        nc.sync.dma_start(g_d[row:row + 1, :], gtmp[:])
    for dstt, row in ((glR, 0), (grR, 1)):
        nc.sync.dma_start(
            dstt[:],
            AP(g_d.tensor, g_d.offset + row * g_d.ap[0][0],
               [[0, 128], [1, 1296]]))

    # root: rsEn = softmax(root_W[0,0:4] + root_b[0:4]) replicated to 4 parts
    rs4 = keep.tile([1, 8], F32)
    rsE = keep.tile([1, 8], F32)
    nc.vector.tensor_tensor(rs4[:, 0:4], smallv[:, 2:6], smallv[:, 6:10],
                            op=ALU.add)
    nc.vector.tensor_reduce(rs4[:, 4:5], rs4[:, 0:4], axis=AXIS.X, op=ALU.max)
    nc.vector.tensor_scalar_mul(rs4[:, 5:6], rs4[:, 4:5], -1.0)
    nc.scalar.activation(rsE[:, 0:4], rs4[:, 0:4], ACTF.Exp,
                         bias=rs4[:, 5:6], accum_out=rsE[:, 4:5])
    nc.vector.reciprocal(rsE[:, 5:6], rsE[:, 4:5])
    nc.vector.tensor_scalar_mul(rsE[:, 0:4], rsE[:, 0:4], rsE[:, 5:6])
    rs_d = dram.tile([1, 4], F32)
    nc.sync.dma_start(rs_d[:], rsE[:, 0:4])
    nc.sync.dma_start(rsRep[:],
                      AP(rs_d.tensor, rs_d.offset, [[0, 4], [1, 4]]))

    # =======================================================================
    # Phase 3: beta1 = wordW.T @ [ntembT; adj] -> exp tables WA/WB
    # =======================================================================
    nc.sync.dma_start(rhs_b[0:65, :], ntembT[:])
    nc.sync.dma_start(rhs_b[65:66, :], adj[:])

    with tc.tile_pool(name="psum_b", bufs=1, space="PSUM") as psb:
        pb = psb.tile([PAIRS, CP], F32)
        for c0 in range(0, CP, 512):
            c1 = min(c0 + 512, CP)
            nc.tensor.matmul(pb[:, c0:c1], wordW[:], rhs_b[:, c0:c1],
                             start=True, stop=True)
        nc.vector.tensor_reduce(M1[:, 0:1], pb[:, 0:C], axis=AXIS.X,
                                op=ALU.max)
        nc.vector.tensor_scalar_mul(M1[:, 1:2], M1[:, 0:1], -1.0)
        nc.scalar.activation(beta1E[:], pb[:], ACTF.Exp, bias=M1[:, 1:2])

    # W tables [PAIRS, 1297]: WB = gather_lf(beta1E)*GrE (+M1), WA = rf/GlE
    for W, op_id, gR in ((WB, 0, grR), (WA, 1, glR)):
        off = lf_block_offsets(op_id)
        blocks = [
            (0, [[36, 4], [1, 4]], off["A"], [[4, 4], [1, 4]]),
            (4, [[36, 4], [1, 32]], off["B"], [[32, 4], [1, 32]]),
            (144, [[1, 1152]], off["C"], [[1, 1152]]),
        ]
        for (oo, od, io, idm) in blocks:
            nc.vector.scalar_tensor_tensor(
                mk(W, PAIRS, oo, od),
                mk(beta1E, PAIRS, io, idm),
                1.0,
                mk(gR, PAIRS, oo, od),
                op0=ALU.mult, op1=ALU.mult)
        nc.vector.tensor_copy(W[:, 1296:1297], M1[:, 0:1])

    # chart block L=1 from beta1E
    nc.vector.tensor_copy(mk(chartA, PAIRS, BLK, [[1, 36]]), beta1E[:, 0:NF])
    nc.vector.tensor_copy(mk(chartA, PAIRS, BLK + 36, [[1, 1]]), M1[:, 0:1])
    nc.vector.tensor_tensor(mk(chartA, PAIRS, BLK + 40, [[4, 4], [1, 4]]),
                            mk(beta1E, PAIRS, 20, [[4, 4], [1, 4]]),
                            mk(glR, PAIRS, 0, [[36, 4], [1, 4]]),
                            op=ALU.mult)
    nc.vector.tensor_tensor(mk(chartA, PAIRS, BLK + 56, [[4, 4], [1, 4]]),
                            mk(beta1E, PAIRS, 4, [[4, 4], [1, 4]]),
                            mk(grR, PAIRS, 0, [[36, 4], [1, 4]]),
                            op=ALU.mult)
    # chartEnd block m lives at col (n-m)*BLK (reversed layout; makes all
    # k-strided reads positive-step). Block 1: end j = i+1 -> row pair.
    nc.sync.dma_start(mk(chartE, PAIRS, (n - 1) * BLK, [[1, BLK]]),
                      mk(chartA, PAIRS, BLK, [[1, BLK]]))

    ph1.close()  # free ph1 tensors before the CKY working set

    es2 = contextlib.ExitStack()
    stage_pool = es2.enter_context(tc.tile_pool(name="stage", bufs=2))
    wash_pool = es2.enter_context(tc.tile_pool(name="wash", bufs=2))
    scr = es2.enter_context(tc.tile_pool(name="cky", bufs=2))
    scr1 = es2.enter_context(tc.tile_pool(name="cky1", bufs=1))

    # =======================================================================
    # Phase 4: CKY in scaled-exp space
    # chartA[pair, L*BLK+.]: 0:36 chartE | 36 scale | 40:56 FA | 56:72 FB
    # chartE[(j-1)*4+b, ...] same, indexed by span end j.
    # =======================================================================
    for L in range(2, n + 1):
        S = n - L + 1
        PS = 4 * S
        NI = L - 2

        stageE = stage_pool.tile([128, n * BLK], F32, tag="st")
        if L >= 3:   # prefetchable part: blocks 1..L-2 (cols (n-m)*BLK)
            nc.sync.dma_start(
                mk(stageE, PS, (n - L + 2) * BLK, [[1, (L - 2) * BLK]]),
                mk(chartE, PS, (n - L + 2) * BLK, [[1, (L - 2) * BLK]],
                   base_part=4 * (L - 1)))
        # critical part: block L-1 = chartA rows [4 .. 4+PS]
        nc.sync.dma_start(
            mk(stageE, PS, (n - L + 1) * BLK, [[1, BLK]]),
            mk(chartA, PS, (L - 1) * BLK, [[1, BLK]], base_part=4))

        wash = wash_pool.tile([128, 1312], F32, tag="wa")
        nc.sync.dma_start(
            mk(wash, PS, 0, [[1, 1297]]),
            mk(WA, PS, 0, [[1, 1297]], base_part=4 * (L - 1)))

        # ---- scales: sAsm = [sB | sA | sI(k=1..L-1)]
        sAsm = scr.tile([128, n + 8], F32, tag="sasm")
        nc.vector.tensor_tensor(
            sAsm[0:PS, 0:1],
            mk(stageE, PS, (n - L + 1) * BLK + 36, [[1, 1]]),
            M1[0:PS, 0:1], op=ALU.add)
        nc.vector.tensor_tensor(
            sAsm[0:PS, 1:2],
            mk(chartA, PS, (L - 1) * BLK + 36, [[1, 1]]),
            mk(wash, PS, 1296, [[1, 1]]), op=ALU.add)
        nc.vector.tensor_tensor(
            sAsm[0:PS, 2:L + 1],
            mk(chartA, PS, BLK + 36, [[BLK, L - 1]]),
            mk(stageE, PS, (n - L + 1) * BLK + 36, [[BLK, L - 1]]),
            op=ALU.add)
        mstar = scr.tile([128, 2], F32, tag="mstar")
        nc.vector.tensor_reduce(mstar[0:PS, 0:1], sAsm[0:PS, 0:L + 1],
                                axis=AXIS.X, op=ALU.max)
        nc.vector.tensor_scalar_mul(mstar[0:PS, 1:2], mstar[0:PS, 0:1], -1.0)
        eAll = scr.tile([128, n + 8], F32, tag="eall")
        nc.scalar.activation(eAll[0:PS, 0:L + 1], sAsm[0:PS, 0:L + 1],
                             ACTF.Exp, bias=mstar[0:PS, 1:2])

        # ---- edge products -> prodAB [PS, 2592], one group-reduce
        prodAB = scr1.tile([128, 2592], F32, tag="prod")
        nc.vector.scalar_tensor_tensor(
            prodAB[0:PS, 0:1296],
            mk(wash, PS, 0, [[1, 1296]]),
            eAll[0:PS, 1:2],
            mk(chartA, PS, (L - 1) * BLK, [[0, 36], [1, 36]]),
            op0=ALU.mult, op1=ALU.mult)
        nc.vector.scalar_tensor_tensor(
            prodAB[0:PS, 1296:2592],
            mk(WB, PS, 0, [[1, 1296]]),
            eAll[0:PS, 0:1],
            mk(stageE, PS, (n - L + 1) * BLK, [[0, 36], [1, 36]]),
            op0=ALU.mult, op1=ALU.mult)
        red72 = scr.tile([128, 72], F32, tag="red")
        nc.vector.tensor_reduce(red72[0:PS, :],
                                mk(prodAB, PS, 0, [[36, 72], [1, 36]]),
                                axis=AXIS.X, op=ALU.add)
        total36 = scr.tile([128, 40], F32, tag="tot")
        nc.vector.tensor_tensor(total36[0:PS, 0:36], red72[0:PS, 0:36],
                                red72[0:PS, 36:72], op=ALU.add)

        # ---- interior terms (res<4), batched over k
        if NI > 0:
            tI = scr1.tile([128, 2 * max(n - 2, 1) * 16], F32, tag="ti")
            nc.vector.tensor_tensor(   # IA: chart[k][i] args x stage FA(L-k)
                mk(tI, PS, 0, [[2 * NI * 4, 4], [4, NI], [1, 4]]),
                mk(chartA, PS, BLK, [[0, 4], [BLK, NI], [1, 4]]),
                mk(stageE, PS, (n - L + 1) * BLK + 40,
                   [[4, 4], [BLK, NI], [1, 4]]),
                op=ALU.mult)
            nc.vector.tensor_tensor(   # IB: stage args(L-k) x chart[k] FB
                mk(tI, PS, NI * 4, [[2 * NI * 4, 4], [4, NI], [1, 4]]),
                mk(stageE, PS, (n - L + 2) * BLK, [[0, 4], [BLK, NI], [1, 4]]),
                mk(chartA, PS, 2 * BLK + 56, [[4, 4], [BLK, NI], [1, 4]]),
                op=ALU.mult)
            for half in range(2):      # x eI (k scales), in place
                nc.vector.tensor_tensor(
                    mk(tI, PS, half * NI * 4,
                       [[2 * NI * 4, 4], [4, NI], [1, 4]]),
                    mk(tI, PS, half * NI * 4,
                       [[2 * NI * 4, 4], [4, NI], [1, 4]]),
                    mk(eAll, PS, 2 + half, [[0, 4], [1, NI], [0, 4]]),
                    op=ALU.mult)
            nc.vector.tensor_reduce(   # sum over (side*k, arg) -> [PS, 4]
                total36[0:PS, 36:40],
                mk(tI, PS, 0,
                   [[2 * NI * 4, 4], [4, 2 * NI], [1, 4]]),
                axis=AXIS.XY, op=ALU.add)
            nc.vector.tensor_tensor(total36[0:PS, 0:4], total36[0:PS, 0:4],
                                    total36[0:PS, 36:40], op=ALU.add)

        # ---- rescale and write chart block L
        mval = scr.tile([128, 4], F32, tag="mval")
        nc.vector.tensor_reduce(mval[0:PS, 0:1], total36[0:PS, 0:36],
                                axis=AXIS.X, op=ALU.max)
        nc.vector.reciprocal(mval[0:PS, 1:2], mval[0:PS, 0:1])
        nc.vector.tensor_scalar_mul(
            mk(chartA, PS, L * BLK, [[1, 36]]),
            total36[0:PS, 0:36], mval[0:PS, 1:2])
        nc.scalar.activation(mval[0:PS, 2:3], mval[0:PS, 0:1], ACTF.Ln)
        nc.vector.tensor_tensor(
            mk(chartA, PS, L * BLK + 36, [[1, 1]]),
            mstar[0:PS, 0:1], mval[0:PS, 2:3], op=ALU.add)
        nc.vector.tensor_tensor(
            mk(chartA, PS, L * BLK + 40, [[4, 4], [1, 4]]),
            mk(chartA, PS, L * BLK + 20, [[4, 4], [1, 4]]),
            mk(glR, PS, 0, [[36, 4], [1, 4]]), op=ALU.mult)
        nc.vector.tensor_tensor(
            mk(chartA, PS, L * BLK + 56, [[4, 4], [1, 4]]),
            mk(chartA, PS, L * BLK + 4, [[4, 4], [1, 4]]),
            mk(grR, PS, 0, [[36, 4], [1, 4]]), op=ALU.mult)
        if L < n:   # chartEnd block L at rows (i+L-1)*4+b, col (n-L)*BLK
            nc.sync.dma_start(
                mk(chartE, PS, (n - L) * BLK, [[1, BLK]],
                   base_part=4 * (L - 1)),
                mk(chartA, PS, L * BLK, [[1, BLK]]))

    # =======================================================================
    # Phase 5: root -> nll per sentence
    # =======================================================================
    nc.vector.tensor_tensor(fin[:, 0:4],
                            mk(chartA, 4, n * BLK, [[1, 4]]),
                            rsRep[:], op=ALU.mult)
    nc.vector.tensor_reduce(fin[:, 4:5], fin[:, 0:4], axis=AXIS.X, op=ALU.add)
    nc.scalar.activation(fin[:, 5:6], fin[:, 4:5], ACTF.Ln)
    nc.vector.scalar_tensor_tensor(
        fin[:, 6:7], fin[:, 5:6], -1.0,
        mk(chartA, 4, n * BLK + 36, [[1, 1]]),
        op0=ALU.mult, op1=ALU.subtract)
    nc.sync.dma_start(d["out"][:], fin[:, 6:7])
    es2.close()
    es.close()


# ============================================================== host wrapper
_PROG_CACHE = {}


def _get_program(cfg: Cfg):
    key = (cfg.n, cfg.v_loc, cfg.n_cores)
    if key not in _PROG_CACHE:
        _PROG_CACHE[key] = build_program(cfg)
    return _PROG_CACHE[key]


def make_inmaps(cfg: Cfg, inputs):
    """Host-side shard/pack of FULL inputs -> per-core DRAM input dicts."""
    x = np.asarray(inputs["x"])
    check_functor_tables(np.asarray(inputs["l_functors"]),
                         np.asarray(inputs["r_functors"]))
    nt_emb = np.asarray(inputs["nt_emb"], np.float32)          # [C, D]
    vocab_W = np.asarray(inputs["vocab_W"], np.float32)        # [D, V]
    vocab_b = np.asarray(inputs["vocab_b"], np.float32)        # [V]

    ntembT = np.zeros((65, CP), np.float32)
    ntembT[0:64, 0:C] = nt_emb.T
    ntembT[64, :] = 1.0

    mlpW = np.zeros((64, 322), np.float32)
    for j, k in enumerate(("sW1", "r1W1", "r1W2", "r2W1", "r2W2")):
        mlpW[:, j * 64:(j + 1) * 64] = np.asarray(inputs[k], np.float32)
    mlpW[:, 320:322] = np.asarray(inputs["sW2"], np.float32)

    mlpB = np.zeros((64, 8), np.float32)
    for j, k in enumerate(("sb1", "r1b1", "r1b2", "r2b1", "r2b2")):
        mlpB[:, j] = np.asarray(inputs[k], np.float32)

    ruleWb = np.zeros((36, 144), np.float32)
    ruleWb[:, 0:72] = np.asarray(inputs["rule_W"], np.float32)
    ruleWb[:, 72:144] = np.tile(
        np.asarray(inputs["rule_b"], np.float32)[None, :], (36, 1))

    smallv = np.zeros((1, 16), np.float32)
    smallv[0, 0:2] = np.asarray(inputs["sb2"], np.float32)
    smallv[0, 2:6] = np.asarray(inputs["root_W"], np.float32)[0, 0:4]
    smallv[0, 6:10] = np.asarray(inputs["root_b"], np.float32)[0:4]

    vs = cfg.v_loc
    in_maps = []
    for core in range(cfg.n_cores):
        vocabW = np.zeros((65, cfg.v_pad), np.float32)
        vocabW[64, :] = NEGB
        vocabW[0:64, 0:vs] = vocab_W[:, core * vs:(core + 1) * vs]
        vocabW[64, 0:vs] = vocab_b[core * vs:(core + 1) * vs]

        words = x[core * BLOC:(core + 1) * BLOC, 0:cfg.n]   # [BLOC, n]
        wid = words.T.reshape(-1)                           # pair = i*4 + b
        wordW = np.zeros((66, cfg.pairs), np.float32)
        wordW[0:64, :] = vocab_W[:, wid]
        wordW[64, :] = vocab_b[wid]
        wordW[65, :] = 1.0

        in_maps.append({
            "ntembT": ntembT, "vocabW": vocabW, "wordW": wordW,
            "mlpW": mlpW, "mlpB": mlpB, "ruleWb": ruleWb, "smallv": smallv,
        })
    return in_maps


def kernel(**inputs) -> np.ndarray:
    cfg = Cfg(n=32, v_loc=V // NCORES, n_cores=NCORES)
    nc = _get_program(cfg)
    in_maps = make_inmaps(cfg, inputs)
    res = bass_utils.run_bass_kernel_spmd(
        nc, in_maps, core_ids=list(range(cfg.n_cores)))
    out = np.concatenate([r["out_nll"].reshape(-1) for r in res.results])
    return out.astype(np.float32)


if __name__ == "__main__":
    from reference import setup_inputs, reference
    inputs = {k: np.asarray(v) for k, v in setup_inputs().items()}
    got = kernel(**inputs)
    exp = np.asarray(reference(**inputs))
    rel = np.max(np.abs(got - exp) / np.maximum(np.abs(exp), 1e-6))
    print("expected:", exp[:8])
    print("got     :", got[:8])
    print("Relative error:", rel)



# revision 15
# speedup vs baseline: 1.2952x; 1.2952x over previous
"""Trainium2 Bass kernel for nn_BasicCGInducer (CKY inside algorithm for a
categorial-grammar inducer).

Strategy (8 NeuronCores):
  - Data-parallel over sentences: core j handles sentences 4j..4j+3.
  - Emission log-partition (the big [C,V] softmax denominator) is
    tensor-parallel over vocab: each core computes sum_v exp(logits) for a
    4000-column V-shard, then one AllReduce of [C] partial sums.
  - Everything else (grammar tables, split-MLP, beta1, CKY) is computed
    per-core on its sentence shard in scaled-exp space (no logsumexp on the
    hot path; per-span running max scales).

kernel(**inputs) takes FULL inputs, shards on host, runs one SPMD bass
program on cores 0-7, and reassembles the [32] output.
"""
import sys
import contextlib

sys.path.insert(0, "/opt/trn_rl_repo")

import numpy as np

import concourse.bass as bass
import concourse.bacc as bacc
import concourse.mybir as mybir
import concourse.tile as tile
from concourse.ap import AP
from concourse import bass_utils

F32 = mybir.dt.float32
F32R = mybir.dt.float32r
BF16 = mybir.dt.bfloat16
I32 = mybir.dt.int32
ALU = mybir.AluOpType
ACTF = mybir.ActivationFunctionType
AXIS = mybir.AxisListType
LN2 = 0.6931471805599453

# ---------------------------------------------------------------- constants
P4 = 4          # primitive cats
NF = 36         # non-functor cats
C = 2596        # total cats
CP = 2688       # padded C (21 * 128)
NT = CP // 128  # 21 c-tiles
D = 64
B = 32          # total sentences
NCORES = 8
BLOC = B // NCORES  # 4 sentences per core
V = 32000
BLK = 80        # per-level block stride in chart tensors
NEGB = -1.0e5   # bias for padded vocab columns


class Cfg:
    def __init__(self, n=32, v_loc=4000, n_cores=8):
        self.n = n                      # sentence length
        self.v_loc = v_loc              # vocab shard per core
        self.v_pad = ((v_loc + 511) // 512) * 512
        self.n_cores = n_cores
        self.pairs = 4 * n              # (i, b) pairs on partitions


# ------------------------------------------------------------ functor maps
def lf_block_offsets(op):
    """c = off + {A: 4r+a | B: 32r+(a-4) | C: 36(r-4)+a} per derivation of
    the deterministic functor-id tables. op=0 -> l_functors, 1 -> r_functors."""
    return {
        "A": 4 + 16 * op,            # res<4, arg<4 : c = A + 4*res + arg
        "B": 36 + 1280 * op,         # res<4, arg>=4: c = B + 32*res + (arg-4)
        "C": 164 + 1280 * op,        # res>=4      : c = C0 + 36*(res-4) + arg
    }


def check_functor_tables(l_functors, r_functors):
    for op, tab in ((0, l_functors), (1, r_functors)):
        off = lf_block_offsets(op)
        exp = np.zeros((NF, NF), np.int64)  # [arg, res]
        for res in range(NF):
            for arg in range(NF):
                if res < P4 and arg < P4:
                    exp[arg, res] = off["A"] + 4 * res + arg
                elif res < P4:
                    exp[arg, res] = off["B"] + 32 * res + (arg - 4)
                else:
                    exp[arg, res] = off["C"] + 36 * (res - 4) + arg
        assert np.array_equal(np.asarray(tab, np.int64), exp), (
            f"functor table structure mismatch (op={op})")


# ---------------------------------------------------------------- AP helper
def mk(t, parts, off, dims, base_part=0):
    """Raw AP on tile t: partition range [base_part, base_part+parts),
    free offset `off` (elements), extra free dims [[step, count], ...]."""
    w = t.ap[0][0]
    return AP(t.tensor, t.offset + base_part * w + off, [[w, parts]] + dims)


# ============================================================ device program
def build_program(cfg: Cfg):
    nc = bacc.Bacc("TRN2", target_bir_lowering=False, debug=False,
                   num_devices=cfg.n_cores)
    d = {
        "ntembT": nc.dram_tensor("ntembT", [65, CP], BF16,
                                 kind="ExternalInput"),
        "vocabW": nc.dram_tensor("vocabW", [65, cfg.v_pad], BF16,
                                 kind="ExternalInput"),
        "wordW": nc.dram_tensor("wordW", [65, cfg.pairs], BF16,
                                kind="ExternalInput"),
        "mlpW": nc.dram_tensor("mlpW", [64, 322], BF16, kind="ExternalInput"),
        "mlpB": nc.dram_tensor("mlpB", [64, 8], F32, kind="ExternalInput"),
        "ruleWb": nc.dram_tensor("ruleWb", [36, 144], F32,
                                 kind="ExternalInput"),
        "smallv": nc.dram_tensor("smallv", [1, 16], F32,
                                 kind="ExternalInput"),
        "out": nc.dram_tensor("out_nll", [BLOC, 1], F32,
                              kind="ExternalOutput"),
    }
    with tile.TileContext(nc) as tc:
        _trace(tc, cfg, d)
    nc.compile()
    return nc


def _trace(tc, cfg, d):
    nc = tc.nc
    n, PAIRS, VP = cfg.n, cfg.pairs, cfg.v_pad
    NV = VP // 512                    # 512-col v-tiles per core
    NHALF = (NV + 3) // 4             # ACT chunks of up to 4 v-tiles
    HW = CP // 2                      # MLP half width (1344)

    es = contextlib.ExitStack()
    keep = es.enter_context(tc.tile_pool(name="keep", bufs=1))
    dram = es.enter_context(tc.tile_pool(name="dram", bufs=1, space="DRAM"))

    # ---------------- long-lived tensors
    chartA = keep.tile([PAIRS, (n + 1) * BLK], F32)
    chartE = keep.tile([PAIRS, (n + 1) * BLK], F32)
    WA = keep.tile([PAIRS, 1312], F32)
    WB = keep.tile([PAIRS, 1312], F32)
    glR = keep.tile([128, 1296], F32)
    grR = keep.tile([128, 1296], F32)
    M1 = keep.tile([PAIRS, 2], F32)
    mlpB = keep.tile([64, 8], F32)
    smallv = keep.tile([1, 16], F32)
    sumexp_parts = keep.tile([128, NT * NHALF], F32)
    sumexp_loc = keep.tile([128, NT], F32)
    sumexp_g = keep.tile([128, NT], F32)
    s0E = keep.tile([1, NF], F32)
    db = keep.tile([1, 2], F32)
    rsRep = keep.tile([4, 4], F32)
    fin = keep.tile([4, 8], F32)

    nc.sync.dma_start(mlpB[:], d["mlpB"][:])
    nc.sync.dma_start(smallv[:], d["smallv"][:])
    nc.gpsimd.memset(chartA[:], 0.0)
    nc.gpsimd.memset(chartE[:], 0.0)

    ph1 = contextlib.ExitStack()
    p1 = ph1.enter_context(tc.tile_pool(name="ph1", bufs=1))
    ntembT = p1.tile([65, CP], BF16)
    vocabW = p1.tile([65, VP], BF16)
    wordW = p1.tile([65, PAIRS], BF16)
    mlpW = p1.tile([64, 322], BF16)
    ruleWb = p1.tile([36, 144], F32)
    adjE = p1.tile([1, CP], F32)      # exp-space split1 factor sigmoid(-y)
    zrec_row = p1.tile([1, CP], F32)  # 1/Z per cat, flattened
    E_row = p1.tile([1, CP], F32)     # sigmoid(-y)/Z
    Erep = p1.tile([PAIRS, CP], F32)
    zrec21 = p1.tile([128, NT], F32)
    beta1E = p1.tile([PAIRS, CP], F32)
    ruleflat = p1.tile([1, 36 * 72], F32)

    nc.sync.dma_start(ntembT[:], d["ntembT"][:])
    nc.sync.dma_start(vocabW[:], d["vocabW"][:])
    nc.sync.dma_start(wordW[:], d["wordW"][:])
    nc.sync.dma_start(mlpW[:], d["mlpW"][:])
    nc.sync.dma_start(ruleWb[:], d["ruleWb"][:])

    # =======================================================================
    # Phase 1: emission partition function (exp in place in PSUM + accum_out)
    # =======================================================================
    with tc.tile_pool(name="psum_e", bufs=2, space="PSUM") as pse, \
         tc.tile_pool(name="scr_e", bufs=2) as scre:
        for ct in range(NT):
            for h in range(NHALF):
                vt0 = h * 4
                nvt = min(4, NV - vt0)
                pt = pse.tile([128, 512 * nvt], F32, tag="pse")
                for vt in range(nvt):
                    nc.tensor.matmul(
                        pt[:, vt * 512:(vt + 1) * 512],
                        ntembT[:, ct * 128:(ct + 1) * 128],
                        vocabW[:, (vt0 + vt) * 512:(vt0 + vt + 1) * 512],
                        start=True, stop=True)
                sce = scre.tile([128, 512 * 4], F32, tag="scre")
                nc.scalar.activation(
                    sce[:, 0:512 * nvt], pt[:], ACTF.Exp,
                    accum_out=sumexp_parts[:, ct * NHALF + h:
                                           ct * NHALF + h + 1])

    if NHALF > 1:
        nc.vector.tensor_reduce(
            sumexp_loc[:],
            mk(sumexp_parts, 128, 0, [[NHALF, NT], [1, NHALF]]),
            axis=AXIS.X, op=ALU.add)
    else:
        nc.vector.tensor_copy(sumexp_loc[:], sumexp_parts[:, 0:NT])

    # AllReduce over cores via DRAM bounce
    cc_in = dram.tile([128, NT], F32)
    cc_out = dram.tile([128, NT], F32)
    nc.sync.dma_start(cc_in[:], sumexp_loc[:])
    nc.gpsimd.collective_compute(
        "AllReduce", ALU.add,
        replica_groups=[list(range(cfg.n_cores))],
        ins=[cc_in[:].opt()], outs=[cc_out[:].opt()])

    # =======================================================================
    # Phase 2: split MLP (transposed layout hT [64, *]), rule tables, root
    # (independent of the AllReduce -> overlaps it)
    # =======================================================================
    nc.vector.tensor_tensor(db[:, 0:1], smallv[:, 0:1], smallv[:, 1:2],
                            op=ALU.subtract)

    with tc.tile_pool(name="mlp", bufs=1) as mlp:
        hA = mlp.tile([64, HW], BF16, tag="hA")
        hB = mlp.tile([64, HW], BF16, tag="hB")
        hC = mlp.tile([64, HW], BF16, tag="hC")
        s_rows = mlp.tile([2, HW], F32, tag="srows")
        w1 = mlp.tile([1, HW], F32, tag="w1")
        w2 = mlp.tile([1, HW], F32, tag="w2")
        w3 = mlp.tile([1, HW], F32, tag="w3")

        for half in range(2):
            base = half * HW

            def dense_relu(dst, col0, rhs, bias_col, res_add=None, rb=0,
                           func=ACTF.Relu):
                with tc.tile_pool(name="psum_m", bufs=2,
                                  space="PSUM") as psm:
                    for c0 in range(0, HW, 512):
                        c1 = min(c0 + 512, HW)
                        pm = psm.tile([64, 512], F32, tag="psm")
                        nc.tensor.matmul(pm[:, 0:c1 - c0],
                                         mlpW[:, col0:col0 + 64],
                                         rhs[0:64, rb + c0:rb + c1],
                                         start=True, stop=True)
                        nc.scalar.activation(
                            dst[:, c0:c1], pm[:, 0:c1 - c0], func,
                            bias=mlpB[:, bias_col:bias_col + 1])
                        if res_add is not None:
                            nc.vector.tensor_tensor(
                                dst[:, c0:c1], dst[:, c0:c1],
                                res_add[:, c0:c1], op=ALU.add)

            dense_relu(hA, 0, ntembT, 0, rb=base,
                       func=ACTF.Identity)           # h1 (linear)
            dense_relu(hB, 64, hA, 1)                   # t = relu(h1 W + b)
            dense_relu(hC, 128, hB, 2, res_add=hA)      # h2
            dense_relu(hB, 192, hC, 3)                  # t2
            dense_relu(hA, 256, hB, 4, res_add=hC)      # h3

            with tc.tile_pool(name="psum_s", bufs=2, space="PSUM") as pss:
                for c0 in range(0, HW, 512):
                    c1 = min(c0 + 512, HW)
                    ps = pss.tile([2, 512], F32, tag="pss")
                    nc.tensor.matmul(ps[:, 0:c1 - c0],
                                     mlpW[:, 320:322],
                                     hA[0:64, c0:c1],
                                     start=True, stop=True)
                    nc.vector.tensor_copy(s_rows[:, c0:c1], ps[:, 0:c1 - c0])

            # d = s0 - s1 (s1 via DMA to partition 0)
            nc.sync.dma_start(w1[:], s_rows[1:2, :])
            nc.vector.tensor_tensor(w2[:], s_rows[0:1, :], w1[:],
                                    op=ALU.subtract)
            y = w2
            nc.vector.tensor_scalar_add(y[:], y[:], db[:, 0:1])
            # exp(split1) = exp(-softplus(y)) = sigmoid(-y)
            nc.scalar.activation(adjE[:, base:base + HW], y[:],
                                 ACTF.Sigmoid, scale=-1.0)
            if half == 0:
                # exp(split0) = exp(-softplus(-y)) = sigmoid(y)
                nc.scalar.activation(s0E[:], y[:, 0:NF], ACTF.Sigmoid)

    # rule tables: softmax over 72 per res row
    rsum = keep.tile([36, 72], F32)
    rmax = keep.tile([36, 2], F32)
    rsumexp = keep.tile([36, 2], F32)
    nc.vector.tensor_tensor(rsum[:], ruleWb[:, 0:72], ruleWb[:, 72:144],
                            op=ALU.add)
    nc.vector.tensor_reduce(rmax[:, 0:1], rsum[:], axis=AXIS.X, op=ALU.max)
    nc.vector.tensor_scalar_mul(rmax[:, 1:2], rmax[:, 0:1], -1.0)
    nc.scalar.activation(rsum[:], rsum[:], ACTF.Exp, bias=rmax[:, 1:2],
                         accum_out=rsumexp[:, 0:1])
    nc.vector.reciprocal(rsumexp[:, 1:2], rsumexp[:, 0:1])
    nc.vector.tensor_scalar_mul(rsum[:], rsum[:], rsumexp[:, 1:2])

    # flatten ruleEn to [1, 2592] via DRAM, then G-flats replicated
    rule_d = dram.tile([36, 72], F32)
    nc.sync.dma_start(rule_d[:], rsum[:])
    nc.sync.dma_start(
        AP(ruleflat.tensor, ruleflat.offset,
           [[ruleflat.ap[0][0], 1], [1, 36 * 72]]),
        rule_d[:])
    g_d = dram.tile([2, 1296], F32)
    gtmp = keep.tile([1, 1296], F32)
    for row, off in ((0, 0), (1, 36)):   # 0: Gl (larg), 1: Gr (rarg)
        nc.vector.tensor_tensor(
            gtmp[:],
            mk(ruleflat, 1, off, [[72, 36], [1, 36]]),
            mk(s0E, 1, 0, [[1, 36], [0, 36]]),
            op=ALU.mult)
        nc.sync.dma_start(g_d[row:row + 1, :], gtmp[:])
    for dstt, row in ((glR, 0), (grR, 1)):
        nc.sync.dma_start(
            dstt[:],
            AP(g_d.tensor, g_d.offset + row * g_d.ap[0][0],
               [[0, 128], [1, 1296]]))

    # root: rsEn = softmax(root_W[0,0:4] + root_b[0:4]) replicated to 4 parts
    rs4 = keep.tile([1, 8], F32)
    rsE = keep.tile([1, 8], F32)
    nc.vector.tensor_tensor(rs4[:, 0:4], smallv[:, 2:6], smallv[:, 6:10],
                            op=ALU.add)
    nc.vector.tensor_reduce(rs4[:, 4:5], rs4[:, 0:4], axis=AXIS.X, op=ALU.max)
    nc.vector.tensor_scalar_mul(rs4[:, 5:6], rs4[:, 4:5], -1.0)
    nc.scalar.activation(rsE[:, 0:4], rs4[:, 0:4], ACTF.Exp,
                         bias=rs4[:, 5:6], accum_out=rsE[:, 4:5])
    nc.vector.reciprocal(rsE[:, 5:6], rsE[:, 4:5])
    nc.vector.tensor_scalar_mul(rsE[:, 0:4], rsE[:, 0:4], rsE[:, 5:6])
    rs_d = dram.tile([1, 4], F32)
    nc.sync.dma_start(rs_d[:], rsE[:, 0:4])
    nc.sync.dma_start(rsRep[:],
                      AP(rs_d.tensor, rs_d.offset, [[0, 4], [1, 4]]))

    # =======================================================================
    # Phase 3: beta1 = wordW.T @ ntembT (no adj row; E-factor applied after
    # the AllReduce lands) -> exp tables WA/WB
    # =======================================================================
    with tc.tile_pool(name="psum_b", bufs=1, space="PSUM") as psb:
        pb = psb.tile([PAIRS, CP], F32)
        for c0 in range(0, CP, 512):
            c1 = min(c0 + 512, CP)
            nc.tensor.matmul(pb[:, c0:c1], wordW[:],
                             ntembT[:, c0:c1],
                             start=True, stop=True)
        nc.vector.tensor_reduce(M1[:, 0:1], pb[:, 0:C], axis=AXIS.X,
                                op=ALU.max)
        nc.vector.tensor_scalar_mul(M1[:, 1:2], M1[:, 0:1], -1.0)
        nc.scalar.activation(beta1E[:], pb[:], ACTF.Exp, bias=M1[:, 1:2])

    # ---- AllReduce-dependent tail: E[c] = sigmoid(-y_c) / Z_c
    nc.sync.dma_start(sumexp_g[:], cc_out[:])
    nc.vector.reciprocal(zrec21[:], sumexp_g[:])
    # rearrange [128, NT] -> [1, CP]  (c = ct*128 + p) via DRAM bounce
    z_d = dram.tile([128, NT], F32)
    nc.sync.dma_start(z_d[:], zrec21[:])
    nc.sync.dma_start(
        AP(zrec_row.tensor, zrec_row.offset,
           [[zrec_row.ap[0][0], 1], [128, NT], [1, 128]]),
        AP(z_d.tensor, z_d.offset, [[z_d.ap[0][0], 1], [1, NT], [NT, 128]]))
    nc.vector.tensor_tensor(E_row[:], adjE[:], zrec_row[:], op=ALU.mult)
    e_d = dram.tile([1, CP], F32)
    nc.sync.dma_start(e_d[:], E_row[:])
    nc.sync.dma_start(Erep[:],
                      AP(e_d.tensor, e_d.offset, [[0, PAIRS], [1, CP]]))
    nc.vector.tensor_tensor(beta1E[:], beta1E[:], Erep[:], op=ALU.mult)

    # W tables [PAIRS, 1297]: WB = gather_lf(beta1E)*GrE (+M1), WA = rf/GlE
    for W, op_id, gR in ((WB, 0, grR), (WA, 1, glR)):
        off = lf_block_offsets(op_id)
        blocks = [
            (0, [[36, 4], [1, 4]], off["A"], [[4, 4], [1, 4]]),
            (4, [[36, 4], [1, 32]], off["B"], [[32, 4], [1, 32]]),
            (144, [[1, 1152]], off["C"], [[1, 1152]]),
        ]
        for (oo, od, io, idm) in blocks:
            nc.vector.scalar_tensor_tensor(
                mk(W, PAIRS, oo, od),
                mk(beta1E, PAIRS, io, idm),
                1.0,
                mk(gR, PAIRS, oo, od),
                op0=ALU.mult, op1=ALU.mult)
        nc.vector.tensor_copy(W[:, 1296:1297], M1[:, 0:1])

    # chart block L=1 from beta1E
    nc.vector.tensor_copy(mk(chartA, PAIRS, BLK, [[1, 36]]), beta1E[:, 0:NF])
    nc.vector.tensor_copy(mk(chartA, PAIRS, BLK + 36, [[1, 1]]), M1[:, 0:1])
    nc.vector.tensor_tensor(mk(chartA, PAIRS, BLK + 40, [[4, 4], [1, 4]]),
                            mk(beta1E, PAIRS, 20, [[4, 4], [1, 4]]),
                            mk(glR, PAIRS, 0, [[36, 4], [1, 4]]),
                            op=ALU.mult)
    nc.vector.tensor_tensor(mk(chartA, PAIRS, BLK + 56, [[4, 4], [1, 4]]),
                            mk(beta1E, PAIRS, 4, [[4, 4], [1, 4]]),
                            mk(grR, PAIRS, 0, [[36, 4], [1, 4]]),
                            op=ALU.mult)
    # chartEnd block m lives at col (n-m)*BLK (reversed layout; makes all
    # k-strided reads positive-step). Block 1: end j = i+1 -> row pair.
    nc.sync.dma_start(mk(chartE, PAIRS, (n - 1) * BLK, [[1, BLK]]),
                      mk(chartA, PAIRS, BLK, [[1, BLK]]))

    ph1.close()  # free ph1 tensors before the CKY working set

    es2 = contextlib.ExitStack()
    stage_pool = es2.enter_context(tc.tile_pool(name="stage", bufs=2))
    wash_pool = es2.enter_context(tc.tile_pool(name="wash", bufs=2))
    scr = es2.enter_context(tc.tile_pool(name="cky", bufs=2))
    scr1 = es2.enter_context(tc.tile_pool(name="cky1", bufs=1))

    # =======================================================================
    # Phase 4: CKY in scaled-exp space
    # chartA[pair, L*BLK+.]: 0:36 chartE | 36 scale | 40:56 FA | 56:72 FB
    # chartE[(j-1)*4+b, ...] same, indexed by span end j.
    # =======================================================================
    for L in range(2, n + 1):
        S = n - L + 1
        PS = 4 * S
        NI = L - 2

        stageE = stage_pool.tile([128, n * BLK], F32, tag="st")
        if L >= 3:   # prefetchable part: blocks 1..L-2 (cols (n-m)*BLK)
            nc.sync.dma_start(
                mk(stageE, PS, (n - L + 2) * BLK, [[1, (L - 2) * BLK]]),
                mk(chartE, PS, (n - L + 2) * BLK, [[1, (L - 2) * BLK]],
                   base_part=4 * (L - 1)))
        # critical part: block L-1 = chartA rows [4 .. 4+PS]
        nc.sync.dma_start(
            mk(stageE, PS, (n - L + 1) * BLK, [[1, BLK]]),
            mk(chartA, PS, (L - 1) * BLK, [[1, BLK]], base_part=4))

        wash = wash_pool.tile([128, 1312], F32, tag="wa")
        nc.sync.dma_start(
            mk(wash, PS, 0, [[1, 1297]]),
            mk(WA, PS, 0, [[1, 1297]], base_part=4 * (L - 1)))

        # ---- scales: sAsm = [sB | sA | sI(k=1..L-1)]
        sAsm = scr.tile([128, n + 8], F32, tag="sasm")
        nc.vector.tensor_tensor(
            sAsm[0:PS, 0:1],
            mk(stageE, PS, (n - L + 1) * BLK + 36, [[1, 1]]),
            M1[0:PS, 0:1], op=ALU.add)
        nc.vector.tensor_tensor(
            sAsm[0:PS, 1:2],
            mk(chartA, PS, (L - 1) * BLK + 36, [[1, 1]]),
            mk(wash, PS, 1296, [[1, 1]]), op=ALU.add)
        nc.vector.tensor_tensor(
            sAsm[0:PS, 2:L + 1],
            mk(chartA, PS, BLK + 36, [[BLK, L - 1]]),
            mk(stageE, PS, (n - L + 1) * BLK + 36, [[BLK, L - 1]]),
            op=ALU.add)
        mstar = scr.tile([128, 2], F32, tag="mstar")
        nc.vector.tensor_reduce(mstar[0:PS, 0:1], sAsm[0:PS, 0:L + 1],
                                axis=AXIS.X, op=ALU.max)
        nc.vector.tensor_scalar_mul(mstar[0:PS, 1:2], mstar[0:PS, 0:1], -1.0)
        eAll = scr.tile([128, n + 8], F32, tag="eall")
        nc.scalar.activation(eAll[0:PS, 0:L + 1], sAsm[0:PS, 0:L + 1],
                             ACTF.Exp, bias=mstar[0:PS, 1:2])

        # ---- edge products -> prodAB [PS, 2592], one group-reduce
        prodAB = scr1.tile([128, 2592], F32, tag="prod")
        nc.vector.scalar_tensor_tensor(
            prodAB[0:PS, 0:1296],
            mk(wash, PS, 0, [[1, 1296]]),
            eAll[0:PS, 1:2],
            mk(chartA, PS, (L - 1) * BLK, [[0, 36], [1, 36]]),
            op0=ALU.mult, op1=ALU.mult)
        nc.vector.scalar_tensor_tensor(
            prodAB[0:PS, 1296:2592],
            mk(WB, PS, 0, [[1, 1296]]),
            eAll[0:PS, 0:1],
            mk(stageE, PS, (n - L + 1) * BLK, [[0, 36], [1, 36]]),
            op0=ALU.mult, op1=ALU.mult)
        red72 = scr.tile([128, 72], F32, tag="red")
        nc.vector.tensor_reduce(red72[0:PS, :],
                                mk(prodAB, PS, 0, [[36, 72], [1, 36]]),
                                axis=AXIS.X, op=ALU.add)
        total36 = scr.tile([128, 40], F32, tag="tot")
        nc.vector.tensor_tensor(total36[0:PS, 0:36], red72[0:PS, 0:36],
                                red72[0:PS, 36:72], op=ALU.add)

        # ---- interior terms (res<4), batched over k
        if NI > 0:
            tI = scr1.tile([128, 2 * max(n - 2, 1) * 16], F32, tag="ti")
            nc.vector.tensor_tensor(   # IA: chart[k][i] args x stage FA(L-k)
                mk(tI, PS, 0, [[2 * NI * 4, 4], [4, NI], [1, 4]]),
                mk(chartA, PS, BLK, [[0, 4], [BLK, NI], [1, 4]]),
                mk(stageE, PS, (n - L + 1) * BLK + 40,
                   [[4, 4], [BLK, NI], [1, 4]]),
                op=ALU.mult)
            nc.vector.tensor_tensor(   # IB: stage args(L-k) x chart[k] FB
                mk(tI, PS, NI * 4, [[2 * NI * 4, 4], [4, NI], [1, 4]]),
                mk(stageE, PS, (n - L + 2) * BLK, [[0, 4], [BLK, NI], [1, 4]]),
                mk(chartA, PS, 2 * BLK + 56, [[4, 4], [BLK, NI], [1, 4]]),
                op=ALU.mult)
            for half in range(2):      # x eI (k scales), in place
                nc.vector.tensor_tensor(
                    mk(tI, PS, half * NI * 4,
                       [[2 * NI * 4, 4], [4, NI], [1, 4]]),
                    mk(tI, PS, half * NI * 4,
                       [[2 * NI * 4, 4], [4, NI], [1, 4]]),
                    mk(eAll, PS, 2 + half, [[0, 4], [1, NI], [0, 4]]),
                    op=ALU.mult)
            nc.vector.tensor_reduce(   # sum over (side*k, arg) -> [PS, 4]
                total36[0:PS, 36:40],
                mk(tI, PS, 0,
                   [[2 * NI * 4, 4], [4, 2 * NI], [1, 4]]),
                axis=AXIS.XY, op=ALU.add)
            nc.vector.tensor_tensor(total36[0:PS, 0:4], total36[0:PS, 0:4],
                                    total36[0:PS, 36:40], op=ALU.add)

        # ---- rescale by a power of 2 near the max (log2-exponent bit
        # tricks on DVE; keeps Exp as the only scalar-engine table in CKY)
        mval = scr.tile([128, 8], F32, tag="mval")
        nc.vector.tensor_reduce(mval[0:PS, 0:1], total36[0:PS, 0:36],
                                axis=AXIS.X, op=ALU.max)
        nc.vector.tensor_scalar(
            mval[0:PS, 4:5].bitcast(I32), mval[0:PS, 0:1].bitcast(I32),
            23, None, op0=ALU.logical_shift_right)          # e = biased exp
        nc.vector.tensor_scalar(
            mval[0:PS, 5:6].bitcast(I32), mval[0:PS, 4:5].bitcast(I32),
            -1, 254, op0=ALU.mult, op1=ALU.add)             # 254 - e
        nc.vector.tensor_scalar(
            mval[0:PS, 6:7].bitcast(I32), mval[0:PS, 5:6].bitcast(I32),
            23, None, op0=ALU.logical_shift_left)           # bits of 2^(127-e)
        nc.vector.tensor_copy(mval[0:PS, 7:8], mval[0:PS, 4:5].bitcast(I32))
        nc.vector.tensor_scalar(
            mval[0:PS, 3:4], mval[0:PS, 7:8],
            127.0, LN2, op0=ALU.subtract, op1=ALU.mult)     # ln(2^(e-127))
        nc.vector.tensor_scalar_mul(
            mk(chartA, PS, L * BLK, [[1, 36]]),
            total36[0:PS, 0:36], mval[0:PS, 6:7])
        nc.vector.tensor_tensor(
            mk(chartA, PS, L * BLK + 36, [[1, 1]]),
            mstar[0:PS, 0:1], mval[0:PS, 3:4], op=ALU.add)
        nc.vector.tensor_tensor(
            mk(chartA, PS, L * BLK + 40, [[4, 4], [1, 4]]),
            mk(chartA, PS, L * BLK + 20, [[4, 4], [1, 4]]),
            mk(glR, PS, 0, [[36, 4], [1, 4]]), op=ALU.mult)
        nc.vector.tensor_tensor(
            mk(chartA, PS, L * BLK + 56, [[4, 4], [1, 4]]),
            mk(chartA, PS, L * BLK + 4, [[4, 4], [1, 4]]),
            mk(grR, PS, 0, [[36, 4], [1, 4]]), op=ALU.mult)
        if L < n:   # chartEnd block L at rows (i+L-1)*4+b, col (n-L)*BLK
            nc.sync.dma_start(
                mk(chartE, PS, (n - L) * BLK, [[1, BLK]],
                   base_part=4 * (L - 1)),
                mk(chartA, PS, L * BLK, [[1, BLK]]))

    # =======================================================================
    # Phase 5: root -> nll per sentence
    # =======================================================================
    nc.vector.tensor_tensor(fin[:, 0:4],
                            mk(chartA, 4, n * BLK, [[1, 4]]),
                            rsRep[:], op=ALU.mult)
    nc.vector.tensor_reduce(fin[:, 4:5], fin[:, 0:4], axis=AXIS.X, op=ALU.add)
    nc.scalar.activation(fin[:, 5:6], fin[:, 4:5], ACTF.Ln)
    nc.vector.scalar_tensor_tensor(
        fin[:, 6:7], fin[:, 5:6], -1.0,
        mk(chartA, 4, n * BLK + 36, [[1, 1]]),
        op0=ALU.mult, op1=ALU.subtract)
    nc.sync.dma_start(d["out"][:], fin[:, 6:7])
    es2.close()
    es.close()


# ============================================================== host wrapper
_PROG_CACHE = {}


def _get_program(cfg: Cfg):
    key = (cfg.n, cfg.v_loc, cfg.n_cores)
    if key not in _PROG_CACHE:
        _PROG_CACHE[key] = build_program(cfg)
    return _PROG_CACHE[key]


def make_inmaps(cfg: Cfg, inputs):
    """Host-side shard/pack of FULL inputs -> per-core DRAM input dicts."""
    x = np.asarray(inputs["x"])
    check_functor_tables(np.asarray(inputs["l_functors"]),
                         np.asarray(inputs["r_functors"]))
    nt_emb = np.asarray(inputs["nt_emb"], np.float32)          # [C, D]
    vocab_W = np.asarray(inputs["vocab_W"], np.float32)        # [D, V]
    vocab_b = np.asarray(inputs["vocab_b"], np.float32)        # [V]

    import ml_dtypes
    bf16 = ml_dtypes.bfloat16

    ntembT = np.zeros((65, CP), np.float32)
    ntembT[0:64, 0:C] = nt_emb.T
    ntembT[64, :] = 1.0
    ntembT = ntembT.astype(bf16)

    mlpW = np.zeros((64, 322), np.float32)
    for j, k in enumerate(("sW1", "r1W1", "r1W2", "r2W1", "r2W2")):
        mlpW[:, j * 64:(j + 1) * 64] = np.asarray(inputs[k], np.float32)
    mlpW[:, 320:322] = np.asarray(inputs["sW2"], np.float32)
    mlpW = mlpW.astype(bf16)

    mlpB = np.zeros((64, 8), np.float32)
    for j, k in enumerate(("sb1", "r1b1", "r1b2", "r2b1", "r2b2")):
        mlpB[:, j] = np.asarray(inputs[k], np.float32)

    ruleWb = np.zeros((36, 144), np.float32)
    ruleWb[:, 0:72] = np.asarray(inputs["rule_W"], np.float32)
    ruleWb[:, 72:144] = np.tile(
        np.asarray(inputs["rule_b"], np.float32)[None, :], (36, 1))

    smallv = np.zeros((1, 16), np.float32)
    smallv[0, 0:2] = np.asarray(inputs["sb2"], np.float32)
    smallv[0, 2:6] = np.asarray(inputs["root_W"], np.float32)[0, 0:4]
    smallv[0, 6:10] = np.asarray(inputs["root_b"], np.float32)[0:4]

    vs = cfg.v_loc
    in_maps = []
    for core in range(cfg.n_cores):
        vocabW = np.zeros((65, cfg.v_pad), np.float32)
        vocabW[64, :] = NEGB
        vocabW[0:64, 0:vs] = vocab_W[:, core * vs:(core + 1) * vs]
        vocabW[64, 0:vs] = vocab_b[core * vs:(core + 1) * vs]
        vocabW = vocabW.astype(bf16)

        words = x[core * BLOC:(core + 1) * BLOC, 0:cfg.n]   # [BLOC, n]
        wid = words.T.reshape(-1)                           # pair = i*4 + b
        wordW = np.zeros((65, cfg.pairs), np.float32)
        wordW[0:64, :] = vocab_W[:, wid]
        wordW[64, :] = vocab_b[wid]
        wordW = wordW.astype(bf16)

        in_maps.append({
            "ntembT": ntembT, "vocabW": vocabW, "wordW": wordW,
            "mlpW": mlpW, "mlpB": mlpB, "ruleWb": ruleWb, "smallv": smallv,
        })
    return in_maps


def kernel(**inputs) -> np.ndarray:
    cfg = Cfg(n=32, v_loc=V // NCORES, n_cores=NCORES)
    nc = _get_program(cfg)
    in_maps = make_inmaps(cfg, inputs)
    res = bass_utils.run_bass_kernel_spmd(
        nc, in_maps, core_ids=list(range(cfg.n_cores)))
    out = np.concatenate([r["out_nll"].reshape(-1) for r in res.results])
    return out.astype(np.float32)


if __name__ == "__main__":
    from reference import setup_inputs, reference
    inputs = {k: np.asarray(v) for k, v in setup_inputs().items()}
    got = kernel(**inputs)
    exp = np.asarray(reference(**inputs))
    rel = np.max(np.abs(got - exp) / np.maximum(np.abs(exp), 1e-6))
    print("expected:", exp[:8])
    print("got     :", got[:8])
    print("Relative error:", rel)



# revision 20
# speedup vs baseline: 1.3183x; 1.0179x over previous
"""Trainium2 Bass kernel for nn_BasicCGInducer (CKY inside algorithm for a
categorial-grammar inducer).

Strategy (8 NeuronCores):
  - Data-parallel over sentences: core j handles sentences 4j..4j+3.
  - Emission log-partition (the big [C,V] softmax denominator) is
    tensor-parallel over vocab: each core computes sum_v exp(logits) for a
    4000-column V-shard, then one AllReduce of [C] partial sums.
  - Everything else (grammar tables, split-MLP, beta1, CKY) is computed
    per-core on its sentence shard in scaled-exp space (no logsumexp on the
    hot path; per-span running max scales).

kernel(**inputs) takes FULL inputs, shards on host, runs one SPMD bass
program on cores 0-7, and reassembles the [32] output.
"""
import sys
import contextlib

sys.path.insert(0, "/opt/trn_rl_repo")

import numpy as np

import concourse.bass as bass
import concourse.bacc as bacc
import concourse.mybir as mybir
import concourse.tile as tile
from concourse.ap import AP
from concourse import bass_utils

F32 = mybir.dt.float32
F32R = mybir.dt.float32r
BF16 = mybir.dt.bfloat16
I32 = mybir.dt.int32
ALU = mybir.AluOpType
ACTF = mybir.ActivationFunctionType
AXIS = mybir.AxisListType
LN2 = 0.6931471805599453

# ---------------------------------------------------------------- constants
P4 = 4          # primitive cats
NF = 36         # non-functor cats
C = 2596        # total cats
CP = 2688       # padded C (21 * 128)
NT = CP // 128  # 21 c-tiles
D = 64
B = 32          # total sentences
NCORES = 8
BLOC = B // NCORES  # 4 sentences per core
V = 32000
BLK2 = 72       # per-level block stride in bf16 chart tensors
NEGB = -1.0e5   # bias for padded vocab columns


class Cfg:
    def __init__(self, n=32, v_loc=4000, n_cores=8):
        self.n = n                      # sentence length
        self.v_loc = v_loc              # vocab shard per core
        self.v_pad = ((v_loc + 511) // 512) * 512
        self.n_cores = n_cores
        self.pairs = 4 * n              # (i, b) pairs on partitions


# ------------------------------------------------------------ functor maps
def lf_block_offsets(op):
    """c = off + {A: 4r+a | B: 32r+(a-4) | C: 36(r-4)+a} per derivation of
    the deterministic functor-id tables. op=0 -> l_functors, 1 -> r_functors."""
    return {
        "A": 4 + 16 * op,            # res<4, arg<4 : c = A + 4*res + arg
        "B": 36 + 1280 * op,         # res<4, arg>=4: c = B + 32*res + (arg-4)
        "C": 164 + 1280 * op,        # res>=4      : c = C0 + 36*(res-4) + arg
    }


def check_functor_tables(l_functors, r_functors):
    for op, tab in ((0, l_functors), (1, r_functors)):
        off = lf_block_offsets(op)
        exp = np.zeros((NF, NF), np.int64)  # [arg, res]
        for res in range(NF):
            for arg in range(NF):
                if res < P4 and arg < P4:
                    exp[arg, res] = off["A"] + 4 * res + arg
                elif res < P4:
                    exp[arg, res] = off["B"] + 32 * res + (arg - 4)
                else:
                    exp[arg, res] = off["C"] + 36 * (res - 4) + arg
        assert np.array_equal(np.asarray(tab, np.int64), exp), (
            f"functor table structure mismatch (op={op})")


# ---------------------------------------------------------------- AP helper
def mk(t, parts, off, dims, base_part=0):
    """Raw AP on tile t: partition range [base_part, base_part+parts),
    free offset `off` (elements), extra free dims [[step, count], ...]."""
    w = t.ap[0][0]
    return AP(t.tensor, t.offset + base_part * w + off, [[w, parts]] + dims)


# ============================================================ device program
def build_program(cfg: Cfg):
    nc = bacc.Bacc("TRN2", target_bir_lowering=False, debug=False,
                   num_devices=cfg.n_cores)
    d = {
        "ntembT": nc.dram_tensor("ntembT", [65, CP], BF16,
                                 kind="ExternalInput"),
        "vocabW": nc.dram_tensor("vocabW", [65, cfg.v_pad], BF16,
                                 kind="ExternalInput"),
        "wordW": nc.dram_tensor("wordW", [65, cfg.pairs], BF16,
                                kind="ExternalInput"),
        "mlpW": nc.dram_tensor("mlpW", [64, 322], BF16, kind="ExternalInput"),
        "mlpB": nc.dram_tensor("mlpB", [64, 8], F32, kind="ExternalInput"),
        "ruleWb": nc.dram_tensor("ruleWb", [36, 144], F32,
                                 kind="ExternalInput"),
        "smallv": nc.dram_tensor("smallv", [1, 16], F32,
                                 kind="ExternalInput"),
        "out": nc.dram_tensor("out_nll", [BLOC, 1], F32,
                              kind="ExternalOutput"),
    }
    with tile.TileContext(nc) as tc:
        _trace(tc, cfg, d)
    nc.compile()
    return nc


def _trace(tc, cfg, d):
    nc = tc.nc
    n, PAIRS, VP = cfg.n, cfg.pairs, cfg.v_pad
    NV = VP // 512                    # 512-col v-tiles per core
    NHALF = (NV + 3) // 4             # ACT chunks of up to 4 v-tiles
    HW = CP // 2                      # MLP half width (1344)

    es = contextlib.ExitStack()
    keep = es.enter_context(tc.tile_pool(name="keep", bufs=1))
    dram = es.enter_context(tc.tile_pool(name="dram", bufs=1, space="DRAM"))

    # ---------------- long-lived tensors
    # chart blocks (bf16 values): 0:36 inside | 36:52 FA | 52:68 FB | pad 4
    chartV = keep.tile([PAIRS, (n + 1) * BLK2], BF16)
    chartS = keep.tile([PAIRS, n + 1], F32)          # log-scale per block
    chartEV = keep.tile([PAIRS, (n + 1) * BLK2], BF16)  # end-indexed, rev
    chartES = keep.tile([PAIRS, n + 1], F32)
    WA = keep.tile([PAIRS, 1296], BF16)
    WB = keep.tile([PAIRS, 1296], BF16)
    glR = keep.tile([128, 1296], BF16)
    grR = keep.tile([128, 1296], BF16)
    M1 = keep.tile([PAIRS, 2], F32)
    mlpB = keep.tile([64, 8], F32)
    smallv = keep.tile([1, 16], F32)
    sumexp_parts = keep.tile([128, NT * NHALF], F32)
    sumexp_loc = keep.tile([128, NT], F32)
    sumexp_g = keep.tile([128, NT], F32)
    s0E = keep.tile([1, NF], F32)
    db = keep.tile([1, 2], F32)
    rsRep = keep.tile([4, 4], F32)
    fin = keep.tile([4, 8], F32)

    nc.sync.dma_start(mlpB[:], d["mlpB"][:])
    nc.sync.dma_start(smallv[:], d["smallv"][:])
    nc.gpsimd.memset(chartV[:], 0.0)
    nc.gpsimd.memset(chartEV[:], 0.0)
    nc.gpsimd.memset(chartS[:], 0.0)
    nc.gpsimd.memset(chartES[:], 0.0)

    ph1 = contextlib.ExitStack()
    p1 = ph1.enter_context(tc.tile_pool(name="ph1", bufs=1))
    ntembT = p1.tile([65, CP], BF16)
    vocabW = p1.tile([65, VP], BF16)
    wordW = p1.tile([65, PAIRS], BF16)
    mlpW = p1.tile([64, 322], BF16)
    ruleWb = p1.tile([36, 144], F32)
    adjE = p1.tile([1, CP], F32)      # exp-space split1 factor sigmoid(-y)
    zrec_row = p1.tile([1, CP], F32)  # 1/Z per cat, flattened
    E_row = p1.tile([1, CP], F32)     # sigmoid(-y)/Z
    E_bf = p1.tile([1, CP], BF16)
    Erep = p1.tile([PAIRS, CP], BF16)
    zrec21 = p1.tile([128, NT], F32)
    beta1E = p1.tile([PAIRS, CP], BF16)
    ruleflat = p1.tile([1, 36 * 72], F32)

    nc.sync.dma_start(ntembT[:], d["ntembT"][:])
    nc.sync.dma_start(vocabW[:], d["vocabW"][:])
    nc.sync.dma_start(wordW[:], d["wordW"][:])
    nc.sync.dma_start(mlpW[:], d["mlpW"][:])
    nc.sync.dma_start(ruleWb[:], d["ruleWb"][:])

    # =======================================================================
    # Phase 1: emission partition function (exp in place in PSUM + accum_out)
    # =======================================================================
    with tc.tile_pool(name="psum_e", bufs=2, space="PSUM") as pse, \
         tc.tile_pool(name="scr_e", bufs=2) as scre:
        for ct in range(NT):
            for h in range(NHALF):
                vt0 = h * 4
                nvt = min(4, NV - vt0)
                pt = pse.tile([128, 512 * nvt], F32, tag="pse")
                for vt in range(nvt):
                    nc.tensor.matmul(
                        pt[:, vt * 512:(vt + 1) * 512],
                        ntembT[:, ct * 128:(ct + 1) * 128],
                        vocabW[:, (vt0 + vt) * 512:(vt0 + vt + 1) * 512],
                        start=True, stop=True)
                sce = scre.tile([128, 512 * 4], F32, tag="scre")
                nc.scalar.activation(
                    sce[:, 0:512 * nvt], pt[:], ACTF.Exp,
                    accum_out=sumexp_parts[:, ct * NHALF + h:
                                           ct * NHALF + h + 1])

    if NHALF > 1:
        nc.vector.tensor_reduce(
            sumexp_loc[:],
            mk(sumexp_parts, 128, 0, [[NHALF, NT], [1, NHALF]]),
            axis=AXIS.X, op=ALU.add)
    else:
        nc.vector.tensor_copy(sumexp_loc[:], sumexp_parts[:, 0:NT])

    # AllReduce over cores via DRAM bounce
    cc_in = dram.tile([128, NT], F32)
    cc_out = dram.tile([128, NT], F32)
    nc.sync.dma_start(cc_in[:], sumexp_loc[:])
    nc.gpsimd.collective_compute(
        "AllReduce", ALU.add,
        replica_groups=[list(range(cfg.n_cores))],
        ins=[cc_in[:].opt()], outs=[cc_out[:].opt()])

    # =======================================================================
    # Phase 2: split MLP (transposed layout hT [64, *]), rule tables, root
    # (independent of the AllReduce -> overlaps it)
    # =======================================================================
    nc.vector.tensor_tensor(db[:, 0:1], smallv[:, 0:1], smallv[:, 1:2],
                            op=ALU.subtract)

    with tc.tile_pool(name="mlp", bufs=1) as mlp:
        hA = mlp.tile([64, HW], BF16, tag="hA")
        hB = mlp.tile([64, HW], BF16, tag="hB")
        hC = mlp.tile([64, HW], BF16, tag="hC")
        s_rows = mlp.tile([2, HW], F32, tag="srows")
        w1 = mlp.tile([1, HW], F32, tag="w1")
        w2 = mlp.tile([1, HW], F32, tag="w2")
        w3 = mlp.tile([1, HW], F32, tag="w3")

        for half in range(2):
            base = half * HW

            def dense_relu(dst, col0, rhs, bias_col, res_add=None, rb=0,
                           func=ACTF.Relu):
                with tc.tile_pool(name="psum_m", bufs=2,
                                  space="PSUM") as psm:
                    for c0 in range(0, HW, 512):
                        c1 = min(c0 + 512, HW)
                        pm = psm.tile([64, 512], F32, tag="psm")
                        nc.tensor.matmul(pm[:, 0:c1 - c0],
                                         mlpW[:, col0:col0 + 64],
                                         rhs[0:64, rb + c0:rb + c1],
                                         start=True, stop=True)
                        nc.scalar.activation(
                            dst[:, c0:c1], pm[:, 0:c1 - c0], func,
                            bias=mlpB[:, bias_col:bias_col + 1])
                        if res_add is not None:
                            nc.vector.tensor_tensor(
                                dst[:, c0:c1], dst[:, c0:c1],
                                res_add[:, c0:c1], op=ALU.add)

            dense_relu(hA, 0, ntembT, 0, rb=base,
                       func=ACTF.Identity)           # h1 (linear)
            dense_relu(hB, 64, hA, 1)                   # t = relu(h1 W + b)
            dense_relu(hC, 128, hB, 2, res_add=hA)      # h2
            dense_relu(hB, 192, hC, 3)                  # t2
            dense_relu(hA, 256, hB, 4, res_add=hC)      # h3

            with tc.tile_pool(name="psum_s", bufs=2, space="PSUM") as pss:
                for c0 in range(0, HW, 512):
                    c1 = min(c0 + 512, HW)
                    ps = pss.tile([2, 512], F32, tag="pss")
                    nc.tensor.matmul(ps[:, 0:c1 - c0],
                                     mlpW[:, 320:322],
                                     hA[0:64, c0:c1],
                                     start=True, stop=True)
                    nc.vector.tensor_copy(s_rows[:, c0:c1], ps[:, 0:c1 - c0])

            # d = s0 - s1 (s1 via DMA to partition 0)
            nc.sync.dma_start(w1[:], s_rows[1:2, :])
            nc.vector.tensor_tensor(w2[:], s_rows[0:1, :], w1[:],
                                    op=ALU.subtract)
            y = w2
            nc.vector.tensor_scalar_add(y[:], y[:], db[:, 0:1])
            # exp(split1) = exp(-softplus(y)) = sigmoid(-y)
            nc.scalar.activation(adjE[:, base:base + HW], y[:],
                                 ACTF.Sigmoid, scale=-1.0)
            if half == 0:
                # exp(split0) = exp(-softplus(-y)) = sigmoid(y)
                nc.scalar.activation(s0E[:], y[:, 0:NF], ACTF.Sigmoid)

    # rule tables: softmax over 72 per res row
    rsum = keep.tile([36, 72], F32)
    rmax = keep.tile([36, 2], F32)
    rsumexp = keep.tile([36, 2], F32)
    nc.vector.tensor_tensor(rsum[:], ruleWb[:, 0:72], ruleWb[:, 72:144],
                            op=ALU.add)
    nc.vector.tensor_reduce(rmax[:, 0:1], rsum[:], axis=AXIS.X, op=ALU.max)
    nc.vector.tensor_scalar_mul(rmax[:, 1:2], rmax[:, 0:1], -1.0)
    nc.scalar.activation(rsum[:], rsum[:], ACTF.Exp, bias=rmax[:, 1:2],
                         accum_out=rsumexp[:, 0:1])
    nc.vector.reciprocal(rsumexp[:, 1:2], rsumexp[:, 0:1])
    nc.vector.tensor_scalar_mul(rsum[:], rsum[:], rsumexp[:, 1:2])

    # flatten ruleEn to [1, 2592] via DRAM, then G-flats replicated
    rule_d = dram.tile([36, 72], F32)
    nc.sync.dma_start(rule_d[:], rsum[:])
    nc.sync.dma_start(
        AP(ruleflat.tensor, ruleflat.offset,
           [[ruleflat.ap[0][0], 1], [1, 36 * 72]]),
        rule_d[:])
    g_d = dram.tile([2, 1296], BF16)
    gtmp = keep.tile([1, 1296], BF16)
    for row, off in ((0, 0), (1, 36)):   # 0: Gl (larg), 1: Gr (rarg)
        nc.vector.tensor_tensor(
            gtmp[:],
            mk(ruleflat, 1, off, [[72, 36], [1, 36]]),
            mk(s0E, 1, 0, [[1, 36], [0, 36]]),
            op=ALU.mult)
        nc.sync.dma_start(g_d[row:row + 1, :], gtmp[:])
    for dstt, row in ((glR, 0), (grR, 1)):
        nc.sync.dma_start(
            dstt[:],
            AP(g_d.tensor, g_d.offset + row * g_d.ap[0][0],
               [[0, 128], [1, 1296]]))

    # root: rsEn = softmax(root_W[0,0:4] + root_b[0:4]) replicated to 4 parts
    rs4 = keep.tile([1, 8], F32)
    rsE = keep.tile([1, 8], F32)
    nc.vector.tensor_tensor(rs4[:, 0:4], smallv[:, 2:6], smallv[:, 6:10],
                            op=ALU.add)
    nc.vector.tensor_reduce(rs4[:, 4:5], rs4[:, 0:4], axis=AXIS.X, op=ALU.max)
    nc.vector.tensor_scalar_mul(rs4[:, 5:6], rs4[:, 4:5], -1.0)
    nc.scalar.activation(rsE[:, 0:4], rs4[:, 0:4], ACTF.Exp,
                         bias=rs4[:, 5:6], accum_out=rsE[:, 4:5])
    nc.vector.reciprocal(rsE[:, 5:6], rsE[:, 4:5])
    nc.vector.tensor_scalar_mul(rsE[:, 0:4], rsE[:, 0:4], rsE[:, 5:6])
    rs_d = dram.tile([1, 4], F32)
    nc.sync.dma_start(rs_d[:], rsE[:, 0:4])
    nc.sync.dma_start(rsRep[:],
                      AP(rs_d.tensor, rs_d.offset, [[0, 4], [1, 4]]))

    # =======================================================================
    # Phase 3: beta1 = wordW.T @ ntembT (no adj row; E-factor applied after
    # the AllReduce lands) -> exp tables WA/WB
    # =======================================================================
    with tc.tile_pool(name="psum_b", bufs=1, space="PSUM") as psb:
        pb = psb.tile([PAIRS, CP], F32)
        for c0 in range(0, CP, 512):
            c1 = min(c0 + 512, CP)
            nc.tensor.matmul(pb[:, c0:c1], wordW[:],
                             ntembT[:, c0:c1],
                             start=True, stop=True)
        nc.vector.tensor_reduce(M1[:, 0:1], pb[:, 0:C], axis=AXIS.X,
                                op=ALU.max)
        nc.vector.tensor_scalar_mul(M1[:, 1:2], M1[:, 0:1], -1.0)
        nc.scalar.activation(beta1E[:], pb[:], ACTF.Exp, bias=M1[:, 1:2])

    # ---- AllReduce-dependent tail: E[c] = sigmoid(-y_c) / Z_c
    nc.sync.dma_start(sumexp_g[:], cc_out[:])
    nc.vector.reciprocal(zrec21[:], sumexp_g[:])
    # rearrange [128, NT] -> [1, CP]  (c = ct*128 + p) via DRAM bounce
    z_d = dram.tile([128, NT], F32)
    nc.sync.dma_start(z_d[:], zrec21[:])
    nc.sync.dma_start(
        AP(zrec_row.tensor, zrec_row.offset,
           [[zrec_row.ap[0][0], 1], [128, NT], [1, 128]]),
        AP(z_d.tensor, z_d.offset, [[z_d.ap[0][0], 1], [1, NT], [NT, 128]]))
    nc.vector.tensor_tensor(E_row[:], adjE[:], zrec_row[:], op=ALU.mult)
    nc.vector.tensor_copy(E_bf[:], E_row[:])
    e_d = dram.tile([1, CP], BF16)
    nc.sync.dma_start(e_d[:], E_bf[:])
    nc.sync.dma_start(Erep[:],
                      AP(e_d.tensor, e_d.offset, [[0, PAIRS], [1, CP]]))
    nc.vector.tensor_tensor(beta1E[:], beta1E[:], Erep[:], op=ALU.mult)

    # W tables [PAIRS, 1296] bf16: WB = gather_lf(beta1E)*GrE, WA = rf/GlE
    for W, op_id, gR in ((WB, 0, grR), (WA, 1, glR)):
        off = lf_block_offsets(op_id)
        blocks = [
            (0, [[36, 4], [1, 4]], off["A"], [[4, 4], [1, 4]]),
            (4, [[36, 4], [1, 32]], off["B"], [[32, 4], [1, 32]]),
            (144, [[1, 1152]], off["C"], [[1, 1152]]),
        ]
        for (oo, od, io, idm) in blocks:
            nc.vector.scalar_tensor_tensor(
                mk(W, PAIRS, oo, od),
                mk(beta1E, PAIRS, io, idm),
                1.0,
                mk(gR, PAIRS, oo, od),
                op0=ALU.mult, op1=ALU.mult)

    # chart block L=1 from beta1E
    nc.vector.tensor_copy(mk(chartV, PAIRS, BLK2, [[1, 36]]), beta1E[:, 0:NF])
    nc.vector.tensor_copy(chartS[:, 1:2], M1[:, 0:1])
    nc.vector.tensor_tensor(mk(chartV, PAIRS, BLK2 + 36, [[4, 4], [1, 4]]),
                            mk(beta1E, PAIRS, 20, [[4, 4], [1, 4]]),
                            mk(glR, PAIRS, 0, [[36, 4], [1, 4]]),
                            op=ALU.mult)
    nc.vector.tensor_tensor(mk(chartV, PAIRS, BLK2 + 52, [[4, 4], [1, 4]]),
                            mk(beta1E, PAIRS, 4, [[4, 4], [1, 4]]),
                            mk(grR, PAIRS, 0, [[36, 4], [1, 4]]),
                            op=ALU.mult)
    # chartEnd block m lives at col (n-m)*BLK2 (reversed layout; makes all
    # k-strided reads positive-step). Block 1: end j = i+1 -> same rows.
    nc.sync.dma_start(mk(chartEV, PAIRS, (n - 1) * BLK2, [[1, BLK2]]),
                      mk(chartV, PAIRS, BLK2, [[1, BLK2]]))
    nc.sync.dma_start(chartES[:, n - 1:n], chartS[:, 1:2])

    ph1.close()  # free ph1 tensors before the CKY working set

    es2 = contextlib.ExitStack()
    stage_pool = es2.enter_context(tc.tile_pool(name="stage", bufs=2))
    wash_pool = es2.enter_context(tc.tile_pool(name="wash", bufs=2))
    scr = es2.enter_context(tc.tile_pool(name="cky", bufs=2))
    scr1 = es2.enter_context(tc.tile_pool(name="cky1", bufs=1))

    # =======================================================================
    # Phase 4: CKY in scaled-exp space (bf16 values, fp32 scales)
    # chartV[pair, L*BLK2+.]: 0:36 inside | 36:52 FA | 52:68 FB
    # chartS[pair, L]: log-scale. chartEV/chartES indexed by span end j,
    # block m at col (n-m)*BLK2 / (n-m).
    # =======================================================================
    NI_MAX = max(n - 2, 1)
    for L in range(2, n + 1):
        S = n - L + 1
        PS = 4 * S
        NI = L - 2

        stageV = stage_pool.tile([128, n * BLK2], BF16, tag="stv")
        stageS = stage_pool.tile([128, n + 1], F32, tag="sts")
        if L >= 3:   # prefetchable part: blocks 1..L-2
            nc.sync.dma_start(
                mk(stageV, PS, (n - L + 2) * BLK2, [[1, (L - 2) * BLK2]]),
                mk(chartEV, PS, (n - L + 2) * BLK2, [[1, (L - 2) * BLK2]],
                   base_part=4 * (L - 1)))
            nc.sync.dma_start(
                mk(stageS, PS, n - L + 2, [[1, L - 2]]),
                mk(chartES, PS, n - L + 2, [[1, L - 2]],
                   base_part=4 * (L - 1)))
        # critical part: block L-1 = chartV rows [4 .. 4+PS]
        nc.sync.dma_start(
            mk(stageV, PS, (n - L + 1) * BLK2, [[1, BLK2]]),
            mk(chartV, PS, (L - 1) * BLK2, [[1, BLK2]], base_part=4))
        nc.sync.dma_start(
            mk(stageS, PS, n - L + 1, [[1, 1]]),
            mk(chartS, PS, L - 1, [[1, 1]], base_part=4))

        wash = wash_pool.tile([128, 1296], BF16, tag="wa")
        washS = wash_pool.tile([128, 1], F32, tag="was")
        nc.sync.dma_start(
            mk(wash, PS, 0, [[1, 1296]]),
            mk(WA, PS, 0, [[1, 1296]], base_part=4 * (L - 1)))
        nc.sync.dma_start(
            mk(washS, PS, 0, [[1, 1]]),
            mk(M1, PS, 0, [[1, 1]], base_part=4 * (L - 1)))

        # ---- scales: sAsm = [sB | sA | sI(k=1..L-1)]
        sAsm = scr.tile([128, n + 8], F32, tag="sasm")
        nc.vector.tensor_tensor(
            sAsm[0:PS, 0:1], mk(stageS, PS, n - L + 1, [[1, 1]]),
            M1[0:PS, 0:1], op=ALU.add)
        nc.vector.tensor_tensor(
            sAsm[0:PS, 1:2], mk(chartS, PS, L - 1, [[1, 1]]),
            washS[0:PS, 0:1], op=ALU.add)
        nc.vector.tensor_tensor(
            sAsm[0:PS, 2:L + 1],
            mk(chartS, PS, 1, [[1, L - 1]]),
            mk(stageS, PS, n - L + 1, [[1, L - 1]]),
            op=ALU.add)
        mstar = scr.tile([128, 2], F32, tag="mstar")
        nc.vector.tensor_reduce(mstar[0:PS, 0:1], sAsm[0:PS, 0:L + 1],
                                axis=AXIS.X, op=ALU.max)
        nc.vector.tensor_scalar_mul(mstar[0:PS, 1:2], mstar[0:PS, 0:1], -1.0)
        eAll = scr.tile([128, n + 8], F32, tag="eall")
        nc.scalar.activation(eAll[0:PS, 0:L + 1], sAsm[0:PS, 0:L + 1],
                             ACTF.Exp, bias=mstar[0:PS, 1:2])

        # ---- edge products -> prodAB bf16 [PS, 2592], tree-fold reduce
        prodAB = scr1.tile([128, 2592], BF16, tag="prod")
        nc.vector.scalar_tensor_tensor(
            prodAB[0:PS, 0:1296],
            mk(wash, PS, 0, [[1, 1296]]),
            eAll[0:PS, 1:2],
            mk(chartV, PS, (L - 1) * BLK2, [[0, 36], [1, 36]]),
            op0=ALU.mult, op1=ALU.mult)
        nc.vector.scalar_tensor_tensor(
            prodAB[0:PS, 1296:2592],
            mk(WB, PS, 0, [[1, 1296]]),
            eAll[0:PS, 0:1],
            mk(stageV, PS, (n - L + 1) * BLK2, [[0, 36], [1, 36]]),
            op0=ALU.mult, op1=ALU.mult)
        f1 = scr.tile([128, 1296], BF16, tag="f1")
        nc.vector.tensor_tensor(f1[0:PS, :], prodAB[0:PS, 0:1296],
                                prodAB[0:PS, 1296:2592], op=ALU.add)
        f2 = scr.tile([128, 648], BF16, tag="f2")
        nc.vector.tensor_tensor(
            mk(f2, PS, 0, [[18, 36], [1, 18]]),
            mk(f1, PS, 0, [[36, 36], [1, 18]]),
            mk(f1, PS, 18, [[36, 36], [1, 18]]), op=ALU.add)
        f3 = scr.tile([128, 324], BF16, tag="f3")
        nc.vector.tensor_tensor(
            mk(f3, PS, 0, [[9, 36], [1, 9]]),
            mk(f2, PS, 0, [[18, 36], [1, 9]]),
            mk(f2, PS, 9, [[18, 36], [1, 9]]), op=ALU.add)
        total36 = scr.tile([128, 40], F32, tag="tot")
        nc.vector.tensor_reduce(total36[0:PS, 0:36],
                                mk(f3, PS, 0, [[9, 36], [1, 9]]),
                                axis=AXIS.X, op=ALU.add)

        # ---- interior terms (res<4), batched over k, eI pre-folded in args
        if NI > 0:
            argsI = scr.tile([128, 8 * NI_MAX], BF16, tag="argsi")
            nc.vector.tensor_tensor(   # left args (chart k) x eI[k]
                mk(argsI, PS, 0, [[4, NI], [1, 4]]),
                mk(chartV, PS, BLK2, [[BLK2, NI], [1, 4]]),
                mk(eAll, PS, 2, [[1, NI], [0, 4]]), op=ALU.mult)
            nc.vector.tensor_tensor(   # right args (stage L-k) x eI[k]
                mk(argsI, PS, 4 * NI_MAX, [[4, NI], [1, 4]]),
                mk(stageV, PS, (n - L + 2) * BLK2, [[BLK2, NI], [1, 4]]),
                mk(eAll, PS, 3, [[1, NI], [0, 4]]), op=ALU.mult)
            tI = scr1.tile([128, 2 * NI_MAX * 16], BF16, tag="ti")
            nc.vector.tensor_tensor(   # IA: scaled left args x stage FA(L-k)
                mk(tI, PS, 0, [[2 * NI * 4, 4], [4, NI], [1, 4]]),
                mk(argsI, PS, 0, [[0, 4], [4, NI], [1, 4]]),
                mk(stageV, PS, (n - L + 1) * BLK2 + 36,
                   [[4, 4], [BLK2, NI], [1, 4]]),
                op=ALU.mult)
            nc.vector.tensor_tensor(   # IB: scaled right args x chart[k] FB
                mk(tI, PS, NI * 4, [[2 * NI * 4, 4], [4, NI], [1, 4]]),
                mk(argsI, PS, 4 * NI_MAX, [[0, 4], [4, NI], [1, 4]]),
                mk(chartV, PS, 2 * BLK2 + 52, [[4, 4], [BLK2, NI], [1, 4]]),
                op=ALU.mult)
            nc.vector.tensor_reduce(   # sum over (side*k, arg) -> [PS, 4]
                total36[0:PS, 36:40],
                mk(tI, PS, 0,
                   [[2 * NI * 4, 4], [4, 2 * NI], [1, 4]]),
                axis=AXIS.XY, op=ALU.add)
            nc.vector.tensor_tensor(total36[0:PS, 0:4], total36[0:PS, 0:4],
                                    total36[0:PS, 36:40], op=ALU.add)

        # ---- rescale by a power of 2 near the max (log2-exponent bit
        # tricks on DVE; keeps Exp as the only scalar-engine table in CKY)
        mval = scr.tile([128, 8], F32, tag="mval")
        nc.vector.tensor_reduce(mval[0:PS, 0:1], total36[0:PS, 0:36],
                                axis=AXIS.X, op=ALU.max)
        nc.vector.tensor_scalar(
            mval[0:PS, 4:5].bitcast(I32), mval[0:PS, 0:1].bitcast(I32),
            23, None, op0=ALU.logical_shift_right)          # e = biased exp
        nc.vector.tensor_scalar(
            mval[0:PS, 5:6].bitcast(I32), mval[0:PS, 4:5].bitcast(I32),
            -1, 254, op0=ALU.mult, op1=ALU.add)             # 254 - e
        nc.vector.tensor_scalar(
            mval[0:PS, 6:7].bitcast(I32), mval[0:PS, 5:6].bitcast(I32),
            23, None, op0=ALU.logical_shift_left)           # bits of 2^(127-e)
        nc.vector.tensor_copy(mval[0:PS, 7:8], mval[0:PS, 4:5].bitcast(I32))
        nc.vector.tensor_scalar(
            mval[0:PS, 3:4], mval[0:PS, 7:8],
            127.0, LN2, op0=ALU.subtract, op1=ALU.mult)     # ln(2^(e-127))
        nc.vector.tensor_scalar_mul(
            mk(chartV, PS, L * BLK2, [[1, 36]]),
            total36[0:PS, 0:36], mval[0:PS, 6:7])
        nc.vector.tensor_tensor(
            mk(chartS, PS, L, [[1, 1]]),
            mstar[0:PS, 0:1], mval[0:PS, 3:4], op=ALU.add)
        nc.vector.tensor_tensor(
            mk(chartV, PS, L * BLK2 + 36, [[4, 4], [1, 4]]),
            mk(chartV, PS, L * BLK2 + 20, [[4, 4], [1, 4]]),
            mk(glR, PS, 0, [[36, 4], [1, 4]]), op=ALU.mult)
        nc.vector.tensor_tensor(
            mk(chartV, PS, L * BLK2 + 52, [[4, 4], [1, 4]]),
            mk(chartV, PS, L * BLK2 + 4, [[4, 4], [1, 4]]),
            mk(grR, PS, 0, [[36, 4], [1, 4]]), op=ALU.mult)
        if L < n:   # chartEnd block L at rows (i+L-1)*4+b
            nc.sync.dma_start(
                mk(chartEV, PS, (n - L) * BLK2, [[1, BLK2]],
                   base_part=4 * (L - 1)),
                mk(chartV, PS, L * BLK2, [[1, BLK2]]))
            nc.sync.dma_start(
                mk(chartES, PS, n - L, [[1, 1]], base_part=4 * (L - 1)),
                mk(chartS, PS, L, [[1, 1]]))

    # =======================================================================
    # Phase 5: root -> nll per sentence
    # =======================================================================
    nc.vector.tensor_tensor(fin[:, 0:4],
                            mk(chartV, 4, n * BLK2, [[1, 4]]),
                            rsRep[:], op=ALU.mult)
    nc.vector.tensor_reduce(fin[:, 4:5], fin[:, 0:4], axis=AXIS.X, op=ALU.add)
    nc.scalar.activation(fin[:, 5:6], fin[:, 4:5], ACTF.Ln)
    nc.vector.scalar_tensor_tensor(
        fin[:, 6:7], fin[:, 5:6], -1.0,
        mk(chartS, 4, n, [[1, 1]]),
        op0=ALU.mult, op1=ALU.subtract)
    nc.sync.dma_start(d["out"][:], fin[:, 6:7])
    es2.close()
    es.close()


# ============================================================== host wrapper
_PROG_CACHE = {}


def _get_program(cfg: Cfg):
    key = (cfg.n, cfg.v_loc, cfg.n_cores)
    if key not in _PROG_CACHE:
        _PROG_CACHE[key] = build_program(cfg)
    return _PROG_CACHE[key]


def make_inmaps(cfg: Cfg, inputs):
    """Host-side shard/pack of FULL inputs -> per-core DRAM input dicts."""
    x = np.asarray(inputs["x"])
    check_functor_tables(np.asarray(inputs["l_functors"]),
                         np.asarray(inputs["r_functors"]))
    nt_emb = np.asarray(inputs["nt_emb"], np.float32)          # [C, D]
    vocab_W = np.asarray(inputs["vocab_W"], np.float32)        # [D, V]
    vocab_b = np.asarray(inputs["vocab_b"], np.float32)        # [V]

    import ml_dtypes
    bf16 = ml_dtypes.bfloat16

    ntembT = np.zeros((65, CP), np.float32)
    ntembT[0:64, 0:C] = nt_emb.T
    ntembT[64, :] = 1.0
    ntembT = ntembT.astype(bf16)

    mlpW = np.zeros((64, 322), np.float32)
    for j, k in enumerate(("sW1", "r1W1", "r1W2", "r2W1", "r2W2")):
        mlpW[:, j * 64:(j + 1) * 64] = np.asarray(inputs[k], np.float32)
    mlpW[:, 320:322] = np.asarray(inputs["sW2"], np.float32)
    mlpW = mlpW.astype(bf16)

    mlpB = np.zeros((64, 8), np.float32)
    for j, k in enumerate(("sb1", "r1b1", "r1b2", "r2b1", "r2b2")):
        mlpB[:, j] = np.asarray(inputs[k], np.float32)

    ruleWb = np.zeros((36, 144), np.float32)
    ruleWb[:, 0:72] = np.asarray(inputs["rule_W"], np.float32)
    ruleWb[:, 72:144] = np.tile(
        np.asarray(inputs["rule_b"], np.float32)[None, :], (36, 1))

    smallv = np.zeros((1, 16), np.float32)
    smallv[0, 0:2] = np.asarray(inputs["sb2"], np.float32)
    smallv[0, 2:6] = np.asarray(inputs["root_W"], np.float32)[0, 0:4]
    smallv[0, 6:10] = np.asarray(inputs["root_b"], np.float32)[0:4]

    vs = cfg.v_loc
    in_maps = []
    for core in range(cfg.n_cores):
        vocabW = np.zeros((65, cfg.v_pad), np.float32)
        vocabW[64, :] = NEGB
        vocabW[0:64, 0:vs] = vocab_W[:, core * vs:(core + 1) * vs]
        vocabW[64, 0:vs] = vocab_b[core * vs:(core + 1) * vs]
        vocabW = vocabW.astype(bf16)

        words = x[core * BLOC:(core + 1) * BLOC, 0:cfg.n]   # [BLOC, n]
        wid = words.T.reshape(-1)                           # pair = i*4 + b
        wordW = np.zeros((65, cfg.pairs), np.float32)
        wordW[0:64, :] = vocab_W[:, wid]
        wordW[64, :] = vocab_b[wid]
        wordW = wordW.astype(bf16)

        in_maps.append({
            "ntembT": ntembT, "vocabW": vocabW, "wordW": wordW,
            "mlpW": mlpW, "mlpB": mlpB, "ruleWb": ruleWb, "smallv": smallv,
        })
    return in_maps


def kernel(**inputs) -> np.ndarray:
    cfg = Cfg(n=32, v_loc=V // NCORES, n_cores=NCORES)
    nc = _get_program(cfg)
    in_maps = make_inmaps(cfg, inputs)
    res = bass_utils.run_bass_kernel_spmd(
        nc, in_maps, core_ids=list(range(cfg.n_cores)))
    out = np.concatenate([r["out_nll"].reshape(-1) for r in res.results])
    return out.astype(np.float32)


if __name__ == "__main__":
    from reference import setup_inputs, reference
    inputs = {k: np.asarray(v) for k, v in setup_inputs().items()}
    got = kernel(**inputs)
    exp = np.asarray(reference(**inputs))
    rel = np.max(np.abs(got - exp) / np.maximum(np.abs(exp), 1e-6))
    print("expected:", exp[:8])
    print("got     :", got[:8])
    print("Relative error:", rel)



# revision 26
# speedup vs baseline: 1.5461x; 1.1728x over previous
"""Trainium2 Bass kernel for nn_BasicCGInducer (CKY inside algorithm for a
categorial-grammar inducer).

Strategy (8 NeuronCores):
  - Data-parallel over sentences: core j handles sentences 4j..4j+3.
  - Emission log-partition (the big [C,V] softmax denominator) is
    tensor-parallel over vocab: each core computes sum_v exp(logits) for a
    4000-column V-shard, then one AllReduce of [C] partial sums.
  - Everything else (grammar tables, split-MLP, beta1, CKY) is computed
    per-core on its sentence shard in scaled-exp space (no logsumexp on the
    hot path; per-span running max scales).

kernel(**inputs) takes FULL inputs, shards on host, runs one SPMD bass
program on cores 0-7, and reassembles the [32] output.
"""
import sys
import contextlib

sys.path.insert(0, "/opt/trn_rl_repo")

import numpy as np

import concourse.bass as bass
import concourse.bacc as bacc
import concourse.mybir as mybir
import concourse.tile as tile
from concourse.ap import AP
from concourse import bass_utils

F32 = mybir.dt.float32
F32R = mybir.dt.float32r
BF16 = mybir.dt.bfloat16
I32 = mybir.dt.int32
ALU = mybir.AluOpType
ACTF = mybir.ActivationFunctionType
AXIS = mybir.AxisListType
LN2 = 0.6931471805599453

# ---------------------------------------------------------------- constants
P4 = 4          # primitive cats
NF = 36         # non-functor cats
C = 2596        # total cats
CP = 2688       # padded C (21 * 128)
NT = CP // 128  # 21 c-tiles
D = 64
B = 32          # total sentences
NCORES = 8
BLOC = B // NCORES  # 4 sentences per core
V = 32000
BLK2 = 72       # per-level block stride in bf16 chart tensors
NEGB = -1.0e5   # bias for padded vocab columns


class Cfg:
    def __init__(self, n=32, v_loc=4000, n_cores=8):
        self.n = n                      # sentence length
        self.v_loc = v_loc              # vocab shard per core
        self.v_pad = ((v_loc + 511) // 512) * 512
        self.n_cores = n_cores
        self.pairs = 4 * n              # (i, b) pairs on partitions


# ------------------------------------------------------------ functor maps
def lf_block_offsets(op):
    """c = off + {A: 4r+a | B: 32r+(a-4) | C: 36(r-4)+a} per derivation of
    the deterministic functor-id tables. op=0 -> l_functors, 1 -> r_functors."""
    return {
        "A": 4 + 16 * op,            # res<4, arg<4 : c = A + 4*res + arg
        "B": 36 + 1280 * op,         # res<4, arg>=4: c = B + 32*res + (arg-4)
        "C": 164 + 1280 * op,        # res>=4      : c = C0 + 36*(res-4) + arg
    }


def check_functor_tables(l_functors, r_functors):
    for op, tab in ((0, l_functors), (1, r_functors)):
        off = lf_block_offsets(op)
        exp = np.zeros((NF, NF), np.int64)  # [arg, res]
        for res in range(NF):
            for arg in range(NF):
                if res < P4 and arg < P4:
                    exp[arg, res] = off["A"] + 4 * res + arg
                elif res < P4:
                    exp[arg, res] = off["B"] + 32 * res + (arg - 4)
                else:
                    exp[arg, res] = off["C"] + 36 * (res - 4) + arg
        assert np.array_equal(np.asarray(tab, np.int64), exp), (
            f"functor table structure mismatch (op={op})")


# ---------------------------------------------------------------- AP helper
def mk(t, parts, off, dims, base_part=0):
    """Raw AP on tile t: partition range [base_part, base_part+parts),
    free offset `off` (elements), extra free dims [[step, count], ...]."""
    w = t.ap[0][0]
    return AP(t.tensor, t.offset + base_part * w + off, [[w, parts]] + dims)


def mkS(t, parts, off, blocks=1, step=72, base_part=0):
    """fp32 view of a pair of bf16 cols at `off` (+k*step) in bf16 tile t."""
    w = t.ap[0][0]
    ap = AP(t.tensor, t.offset + base_part * w + off,
            [[w, parts], [step, blocks], [1, 2]])
    return ap.bitcast(mybir.dt.float32)


# ============================================================ device program
def build_program(cfg: Cfg):
    nc = bacc.Bacc("TRN2", target_bir_lowering=False, debug=False,
                   num_devices=cfg.n_cores)
    d = {
        "ntembT": nc.dram_tensor("ntembT", [65, CP], BF16,
                                 kind="ExternalInput"),
        "vocabW": nc.dram_tensor("vocabW", [65, cfg.v_pad], BF16,
                                 kind="ExternalInput"),
        "wordW": nc.dram_tensor("wordW", [65, cfg.pairs], BF16,
                                kind="ExternalInput"),
        "mlpW": nc.dram_tensor("mlpW", [64, 322], BF16, kind="ExternalInput"),
        "mlpB": nc.dram_tensor("mlpB", [64, 8], F32, kind="ExternalInput"),
        "ruleWb": nc.dram_tensor("ruleWb", [36, 144], F32,
                                 kind="ExternalInput"),
        "smallv": nc.dram_tensor("smallv", [1, 16], F32,
                                 kind="ExternalInput"),
        "out": nc.dram_tensor("out_nll", [BLOC, 1], F32,
                              kind="ExternalOutput"),
    }
    with tile.TileContext(nc) as tc:
        _trace(tc, cfg, d)
    nc.compile()
    return nc


def _trace(tc, cfg, d):
    nc = tc.nc
    n, PAIRS, VP = cfg.n, cfg.pairs, cfg.v_pad
    NV = VP // 512                    # 512-col v-tiles per core
    NHALF = (NV + 3) // 4             # ACT chunks of up to 4 v-tiles
    HW = CP // 2                      # MLP half width (1344)

    es = contextlib.ExitStack()
    keep = es.enter_context(tc.tile_pool(name="keep", bufs=1))
    dram = es.enter_context(tc.tile_pool(name="dram", bufs=1, space="DRAM"))

    # ---------------- long-lived tensors
    # chart blocks (bf16 values): 0:36 inside | 36:52 FA | 52:68 FB | pad 4
    chartV = keep.tile([PAIRS, (n + 1) * BLK2], BF16)
    chartEV = keep.tile([PAIRS, (n + 1) * BLK2], BF16)  # end-indexed, rev
    WA = keep.tile([PAIRS, 1300], BF16)   # cols 1296:1298 = M1 (f32 bits)
    WB = keep.tile([PAIRS, 1300], BF16)
    glR = keep.tile([128, 1296], BF16)
    grR = keep.tile([128, 1296], BF16)
    M1 = keep.tile([PAIRS, 2], F32)
    mlpB = keep.tile([64, 8], F32)
    smallv = keep.tile([1, 16], F32)
    sumexp_parts = keep.tile([128, NT * NHALF], F32)
    sumexp_loc = keep.tile([128, NT], F32)
    sumexp_g = keep.tile([128, NT], F32)
    s0E = keep.tile([1, NF], F32)
    db = keep.tile([1, 2], F32)
    rsRep = keep.tile([4, 4], F32)
    fin = keep.tile([4, 8], F32)

    nc.sync.dma_start(mlpB[:], d["mlpB"][:])
    nc.sync.dma_start(smallv[:], d["smallv"][:])
    nc.gpsimd.memset(chartV[:], 0.0)
    nc.gpsimd.memset(chartEV[:], 0.0)

    ph1 = contextlib.ExitStack()
    p1 = ph1.enter_context(tc.tile_pool(name="ph1", bufs=1))
    ntembT = p1.tile([65, CP], BF16)
    vocabW = p1.tile([65, VP], BF16)
    wordW = p1.tile([65, PAIRS], BF16)
    mlpW = p1.tile([64, 322], BF16)
    ruleWb = p1.tile([36, 144], F32)
    adjE = p1.tile([1, CP], F32)      # exp-space split1 factor sigmoid(-y)
    zrec_row = p1.tile([1, CP], F32)  # 1/Z per cat, flattened
    E_row = p1.tile([1, CP], F32)     # sigmoid(-y)/Z
    E_bf = p1.tile([1, CP], BF16)
    Erep = p1.tile([PAIRS, CP], BF16)
    zrec21 = p1.tile([128, NT], F32)
    beta1E = p1.tile([PAIRS, CP], BF16)
    ruleflat = p1.tile([1, 36 * 72], F32)

    nc.sync.dma_start(ntembT[:], d["ntembT"][:])
    nc.sync.dma_start(vocabW[:], d["vocabW"][:])
    nc.sync.dma_start(wordW[:], d["wordW"][:])
    nc.sync.dma_start(mlpW[:], d["mlpW"][:])
    nc.sync.dma_start(ruleWb[:], d["ruleWb"][:])

    # =======================================================================
    # Phase 1: emission partition function (exp in place in PSUM + accum_out)
    # =======================================================================
    with tc.tile_pool(name="psum_e", bufs=2, space="PSUM") as pse, \
         tc.tile_pool(name="scr_e", bufs=2) as scre:
        for ct in range(NT):
            for h in range(NHALF):
                vt0 = h * 4
                nvt = min(4, NV - vt0)
                pt = pse.tile([128, 512 * nvt], F32, tag="pse")
                for vt in range(nvt):
                    nc.tensor.matmul(
                        pt[:, vt * 512:(vt + 1) * 512],
                        ntembT[:, ct * 128:(ct + 1) * 128],
                        vocabW[:, (vt0 + vt) * 512:(vt0 + vt + 1) * 512],
                        start=True, stop=True)
                sce = scre.tile([128, 512 * 4], F32, tag="scre")
                nc.scalar.activation(
                    sce[:, 0:512 * nvt], pt[:], ACTF.Exp,
                    accum_out=sumexp_parts[:, ct * NHALF + h:
                                           ct * NHALF + h + 1])

    if NHALF > 1:
        nc.vector.tensor_reduce(
            sumexp_loc[:],
            mk(sumexp_parts, 128, 0, [[NHALF, NT], [1, NHALF]]),
            axis=AXIS.X, op=ALU.add)
    else:
        nc.vector.tensor_copy(sumexp_loc[:], sumexp_parts[:, 0:NT])

    # AllReduce over cores via DRAM bounce
    cc_in = dram.tile([128, NT], F32)
    cc_out = dram.tile([128, NT], F32)
    nc.sync.dma_start(cc_in[:], sumexp_loc[:])
    nc.gpsimd.collective_compute(
        "AllReduce", ALU.add,
        replica_groups=[list(range(cfg.n_cores))],
        ins=[cc_in[:].opt()], outs=[cc_out[:].opt()])

    # =======================================================================
    # Phase 2: split MLP (transposed layout hT [64, *]), rule tables, root
    # (independent of the AllReduce -> overlaps it)
    # =======================================================================
    nc.vector.tensor_tensor(db[:, 0:1], smallv[:, 0:1], smallv[:, 1:2],
                            op=ALU.subtract)

    with tc.tile_pool(name="mlp", bufs=1) as mlp:
        hA = mlp.tile([64, HW], BF16, tag="hA")
        hB = mlp.tile([64, HW], BF16, tag="hB")
        hC = mlp.tile([64, HW], BF16, tag="hC")
        s_rows = mlp.tile([2, HW], F32, tag="srows")
        w1 = mlp.tile([1, HW], F32, tag="w1")
        w2 = mlp.tile([1, HW], F32, tag="w2")
        w3 = mlp.tile([1, HW], F32, tag="w3")

        for half in range(2):
            base = half * HW

            def dense_relu(dst, col0, rhs, bias_col, res_add=None, rb=0,
                           func=ACTF.Relu):
                with tc.tile_pool(name="psum_m", bufs=2,
                                  space="PSUM") as psm:
                    for c0 in range(0, HW, 512):
                        c1 = min(c0 + 512, HW)
                        pm = psm.tile([64, 512], F32, tag="psm")
                        nc.tensor.matmul(pm[:, 0:c1 - c0],
                                         mlpW[:, col0:col0 + 64],
                                         rhs[0:64, rb + c0:rb + c1],
                                         start=True, stop=True)
                        nc.scalar.activation(
                            dst[:, c0:c1], pm[:, 0:c1 - c0], func,
                            bias=mlpB[:, bias_col:bias_col + 1])
                        if res_add is not None:
                            nc.vector.tensor_tensor(
                                dst[:, c0:c1], dst[:, c0:c1],
                                res_add[:, c0:c1], op=ALU.add)

            dense_relu(hA, 0, ntembT, 0, rb=base,
                       func=ACTF.Identity)           # h1 (linear)
            dense_relu(hB, 64, hA, 1)                   # t = relu(h1 W + b)
            dense_relu(hC, 128, hB, 2, res_add=hA)      # h2
            dense_relu(hB, 192, hC, 3)                  # t2
            dense_relu(hA, 256, hB, 4, res_add=hC)      # h3

            with tc.tile_pool(name="psum_s", bufs=2, space="PSUM") as pss:
                for c0 in range(0, HW, 512):
                    c1 = min(c0 + 512, HW)
                    ps = pss.tile([2, 512], F32, tag="pss")
                    nc.tensor.matmul(ps[:, 0:c1 - c0],
                                     mlpW[:, 320:322],
                                     hA[0:64, c0:c1],
                                     start=True, stop=True)
                    nc.vector.tensor_copy(s_rows[:, c0:c1], ps[:, 0:c1 - c0])

            # d = s0 - s1 (s1 via DMA to partition 0)
            nc.sync.dma_start(w1[:], s_rows[1:2, :])
            nc.vector.tensor_tensor(w2[:], s_rows[0:1, :], w1[:],
                                    op=ALU.subtract)
            y = w2
            nc.vector.tensor_scalar_add(y[:], y[:], db[:, 0:1])
            # exp(split1) = exp(-softplus(y)) = sigmoid(-y)
            nc.scalar.activation(adjE[:, base:base + HW], y[:],
                                 ACTF.Sigmoid, scale=-1.0)
            if half == 0:
                # exp(split0) = exp(-softplus(-y)) = sigmoid(y)
                nc.scalar.activation(s0E[:], y[:, 0:NF], ACTF.Sigmoid)

    # rule tables: softmax over 72 per res row
    rsum = keep.tile([36, 72], F32)
    rmax = keep.tile([36, 2], F32)
    rsumexp = keep.tile([36, 2], F32)
    nc.vector.tensor_tensor(rsum[:], ruleWb[:, 0:72], ruleWb[:, 72:144],
                            op=ALU.add)
    nc.vector.tensor_reduce(rmax[:, 0:1], rsum[:], axis=AXIS.X, op=ALU.max)
    nc.vector.tensor_scalar_mul(rmax[:, 1:2], rmax[:, 0:1], -1.0)
    nc.scalar.activation(rsum[:], rsum[:], ACTF.Exp, bias=rmax[:, 1:2],
                         accum_out=rsumexp[:, 0:1])
    nc.vector.reciprocal(rsumexp[:, 1:2], rsumexp[:, 0:1])
    nc.vector.tensor_scalar_mul(rsum[:], rsum[:], rsumexp[:, 1:2])

    # flatten ruleEn to [1, 2592] via DRAM, then G-flats replicated
    rule_d = dram.tile([36, 72], F32)
    nc.sync.dma_start(rule_d[:], rsum[:])
    nc.sync.dma_start(
        AP(ruleflat.tensor, ruleflat.offset,
           [[ruleflat.ap[0][0], 1], [1, 36 * 72]]),
        rule_d[:])
    g_d = dram.tile([2, 1296], BF16)
    gtmp = keep.tile([1, 1296], BF16)
    for row, off in ((0, 0), (1, 36)):   # 0: Gl (larg), 1: Gr (rarg)
        nc.vector.tensor_tensor(
            gtmp[:],
            mk(ruleflat, 1, off, [[72, 36], [1, 36]]),
            mk(s0E, 1, 0, [[1, 36], [0, 36]]),
            op=ALU.mult)
        nc.sync.dma_start(g_d[row:row + 1, :], gtmp[:])
    for dstt, row in ((glR, 0), (grR, 1)):
        nc.sync.dma_start(
            dstt[:],
            AP(g_d.tensor, g_d.offset + row * g_d.ap[0][0],
               [[0, 128], [1, 1296]]))

    # root: rsEn = softmax(root_W[0,0:4] + root_b[0:4]) replicated to 4 parts
    rs4 = keep.tile([1, 8], F32)
    rsE = keep.tile([1, 8], F32)
    nc.vector.tensor_tensor(rs4[:, 0:4], smallv[:, 2:6], smallv[:, 6:10],
                            op=ALU.add)
    nc.vector.tensor_reduce(rs4[:, 4:5], rs4[:, 0:4], axis=AXIS.X, op=ALU.max)
    nc.vector.tensor_scalar_mul(rs4[:, 5:6], rs4[:, 4:5], -1.0)
    nc.scalar.activation(rsE[:, 0:4], rs4[:, 0:4], ACTF.Exp,
                         bias=rs4[:, 5:6], accum_out=rsE[:, 4:5])
    nc.vector.reciprocal(rsE[:, 5:6], rsE[:, 4:5])
    nc.vector.tensor_scalar_mul(rsE[:, 0:4], rsE[:, 0:4], rsE[:, 5:6])
    rs_d = dram.tile([1, 4], F32)
    nc.sync.dma_start(rs_d[:], rsE[:, 0:4])
    nc.sync.dma_start(rsRep[:],
                      AP(rs_d.tensor, rs_d.offset, [[0, 4], [1, 4]]))

    # =======================================================================
    # Phase 3: beta1 = wordW.T @ ntembT (no adj row; E-factor applied after
    # the AllReduce lands) -> exp tables WA/WB
    # =======================================================================
    with tc.tile_pool(name="psum_b", bufs=1, space="PSUM") as psb:
        pb = psb.tile([PAIRS, CP], F32)
        for c0 in range(0, CP, 512):
            c1 = min(c0 + 512, CP)
            nc.tensor.matmul(pb[:, c0:c1], wordW[:],
                             ntembT[:, c0:c1],
                             start=True, stop=True)
        nc.vector.tensor_reduce(M1[:, 0:1], pb[:, 0:C], axis=AXIS.X,
                                op=ALU.max)
        nc.vector.tensor_scalar_mul(M1[:, 1:2], M1[:, 0:1], -1.0)
        nc.scalar.activation(beta1E[:], pb[:], ACTF.Exp, bias=M1[:, 1:2])

    # ---- AllReduce-dependent tail: E[c] = sigmoid(-y_c) / Z_c
    nc.sync.dma_start(sumexp_g[:], cc_out[:])
    nc.vector.reciprocal(zrec21[:], sumexp_g[:])
    # rearrange [128, NT] -> [1, CP]  (c = ct*128 + p) via DRAM bounce
    z_d = dram.tile([128, NT], F32)
    nc.sync.dma_start(z_d[:], zrec21[:])
    nc.sync.dma_start(
        AP(zrec_row.tensor, zrec_row.offset,
           [[zrec_row.ap[0][0], 1], [128, NT], [1, 128]]),
        AP(z_d.tensor, z_d.offset, [[z_d.ap[0][0], 1], [1, NT], [NT, 128]]))
    nc.vector.tensor_tensor(E_row[:], adjE[:], zrec_row[:], op=ALU.mult)
    nc.vector.tensor_copy(E_bf[:], E_row[:])
    e_d = dram.tile([1, CP], BF16)
    nc.sync.dma_start(e_d[:], E_bf[:])
    nc.sync.dma_start(Erep[:],
                      AP(e_d.tensor, e_d.offset, [[0, PAIRS], [1, CP]]))
    nc.vector.tensor_tensor(beta1E[:], beta1E[:], Erep[:], op=ALU.mult)

    # W tables [PAIRS, 1296] bf16: WB = gather_lf(beta1E)*GrE, WA = rf/GlE
    for W, op_id, gR in ((WB, 0, grR), (WA, 1, glR)):
        off = lf_block_offsets(op_id)
        blocks = [
            (0, [[36, 4], [1, 4]], off["A"], [[4, 4], [1, 4]]),
            (4, [[36, 4], [1, 32]], off["B"], [[32, 4], [1, 32]]),
            (144, [[1, 1152]], off["C"], [[1, 1152]]),
        ]
        for (oo, od, io, idm) in blocks:
            nc.vector.scalar_tensor_tensor(
                mk(W, PAIRS, oo, od),
                mk(beta1E, PAIRS, io, idm),
                1.0,
                mk(gR, PAIRS, oo, od),
                op0=ALU.mult, op1=ALU.mult)
    nc.vector.tensor_copy(mkS(WA, PAIRS, 1296), M1[:, 0:1])

    # chart block L=1 from beta1E
    nc.vector.tensor_copy(mk(chartV, PAIRS, BLK2, [[1, 36]]), beta1E[:, 0:NF])
    nc.vector.tensor_copy(mkS(chartV, PAIRS, BLK2 + 68), M1[:, 0:1])
    nc.vector.tensor_tensor(mk(chartV, PAIRS, BLK2 + 36, [[4, 4], [1, 4]]),
                            mk(beta1E, PAIRS, 20, [[4, 4], [1, 4]]),
                            mk(glR, PAIRS, 0, [[36, 4], [1, 4]]),
                            op=ALU.mult)
    nc.vector.tensor_tensor(mk(chartV, PAIRS, BLK2 + 52, [[4, 4], [1, 4]]),
                            mk(beta1E, PAIRS, 4, [[4, 4], [1, 4]]),
                            mk(grR, PAIRS, 0, [[36, 4], [1, 4]]),
                            op=ALU.mult)
    # chartEnd block m lives at col (n-m)*BLK2 (reversed layout; makes all
    # k-strided reads positive-step). Block 1: end j = i+1 -> same rows.
    nc.sync.dma_start(mk(chartEV, PAIRS, (n - 1) * BLK2, [[1, BLK2]]),
                      mk(chartV, PAIRS, BLK2, [[1, BLK2]]))

    ph1.close()  # free ph1 tensors before the CKY working set

    es2 = contextlib.ExitStack()
    stage_pool = es2.enter_context(tc.tile_pool(name="stage", bufs=2))
    wash_pool = es2.enter_context(tc.tile_pool(name="wash", bufs=2))
    scr = es2.enter_context(tc.tile_pool(name="cky", bufs=2))
    scr1 = es2.enter_context(tc.tile_pool(name="cky1", bufs=1))

    # =======================================================================
    # Phase 4: CKY in scaled-exp space (bf16 values, fp32 scales)
    # chartV[pair, L*BLK2+.]: 0:36 inside | 36:52 FA | 52:68 FB
    # scale (f32 in bf16 cols 68:70). chartEV end-indexed by span end j,
    # block m at col (n-m)*BLK2.
    # =======================================================================
    NI_MAX = max(n - 2, 1)
    for L in range(2, n + 1):
        S = n - L + 1
        PS = 4 * S
        NI = L - 2

        stageV = stage_pool.tile([128, n * BLK2], BF16, tag="stv")
        if L >= 3:   # prefetchable part: blocks 1..L-2
            nc.sync.dma_start(
                mk(stageV, PS, (n - L + 2) * BLK2, [[1, (L - 2) * BLK2]]),
                mk(chartEV, PS, (n - L + 2) * BLK2, [[1, (L - 2) * BLK2]],
                   base_part=4 * (L - 1)))
        # critical part: block L-1 = chartV rows [4 .. 4+PS]
        nc.sync.dma_start(
            mk(stageV, PS, (n - L + 1) * BLK2, [[1, BLK2]]),
            mk(chartV, PS, (L - 1) * BLK2, [[1, BLK2]], base_part=4))

        wash = wash_pool.tile([128, 1300], BF16, tag="wa")
        nc.sync.dma_start(
            mk(wash, PS, 0, [[1, 1300]]),
            mk(WA, PS, 0, [[1, 1300]], base_part=4 * (L - 1)))

        # ---- scales: sAsm = [sB | sA | sI(k=1..L-1)]
        sAsm = scr.tile([128, n + 8], F32, tag="sasm")
        nc.vector.tensor_tensor(
            sAsm[0:PS, 0:1], mkS(stageV, PS, (n - L + 1) * BLK2 + 68),
            M1[0:PS, 0:1], op=ALU.add)
        nc.vector.tensor_tensor(
            sAsm[0:PS, 1:2], mkS(chartV, PS, (L - 1) * BLK2 + 68),
            mkS(wash, PS, 1296), op=ALU.add)
        nc.vector.tensor_tensor(
            sAsm[0:PS, 2:L + 1],
            mkS(chartV, PS, BLK2 + 68, blocks=L - 1),
            mkS(stageV, PS, (n - L + 1) * BLK2 + 68, blocks=L - 1),
            op=ALU.add)
        mstar = scr.tile([128, 2], F32, tag="mstar")
        nc.vector.tensor_reduce(mstar[0:PS, 0:1], sAsm[0:PS, 0:L + 1],
                                axis=AXIS.X, op=ALU.max)
        nc.vector.tensor_scalar_mul(mstar[0:PS, 1:2], mstar[0:PS, 0:1], -1.0)
        eAll = scr.tile([128, n + 8], F32, tag="eall")
        nc.scalar.activation(eAll[0:PS, 0:L + 1], sAsm[0:PS, 0:L + 1],
                             ACTF.Exp, bias=mstar[0:PS, 1:2])

        # ---- edge products -> prodAB bf16 [PS, 2592], tree-fold reduce
        prodAB = scr1.tile([128, 2592], BF16, tag="prod")
        nc.vector.scalar_tensor_tensor(
            prodAB[0:PS, 0:1296],
            mk(wash, PS, 0, [[1, 1296]]),
            eAll[0:PS, 1:2],
            mk(chartV, PS, (L - 1) * BLK2, [[0, 36], [1, 36]]),
            op0=ALU.mult, op1=ALU.mult)
        nc.vector.scalar_tensor_tensor(
            prodAB[0:PS, 1296:2592],
            mk(WB, PS, 0, [[1, 1296]]),
            eAll[0:PS, 0:1],
            mk(stageV, PS, (n - L + 1) * BLK2, [[0, 36], [1, 36]]),
            op0=ALU.mult, op1=ALU.mult)
        red72 = scr.tile([128, 72], F32, tag="red")
        nc.vector.tensor_reduce(red72[0:PS, :],
                                mk(prodAB, PS, 0, [[36, 72], [1, 36]]),
                                axis=AXIS.X, op=ALU.add)
        total36 = scr.tile([128, 40], F32, tag="tot")
        nc.vector.tensor_tensor(total36[0:PS, 0:36], red72[0:PS, 0:36],
                                red72[0:PS, 36:72], op=ALU.add)

        # ---- interior terms (res<4), batched over k, eI pre-folded in args
        if NI > 0:
            argsI = scr.tile([128, 8 * NI_MAX], BF16, tag="argsi")
            nc.vector.tensor_tensor(   # left args (chart k) x eI[k]
                mk(argsI, PS, 0, [[4, NI], [1, 4]]),
                mk(chartV, PS, BLK2, [[BLK2, NI], [1, 4]]),
                mk(eAll, PS, 2, [[1, NI], [0, 4]]), op=ALU.mult)
            nc.vector.tensor_tensor(   # right args (stage L-k) x eI[k]
                mk(argsI, PS, 4 * NI_MAX, [[4, NI], [1, 4]]),
                mk(stageV, PS, (n - L + 2) * BLK2, [[BLK2, NI], [1, 4]]),
                mk(eAll, PS, 3, [[1, NI], [0, 4]]), op=ALU.mult)
            tI = scr1.tile([128, 2 * NI_MAX * 16], BF16, tag="ti")
            nc.vector.tensor_tensor(   # IA: scaled left args x stage FA(L-k)
                mk(tI, PS, 0, [[2 * NI * 4, 4], [4, NI], [1, 4]]),
                mk(argsI, PS, 0, [[0, 4], [4, NI], [1, 4]]),
                mk(stageV, PS, (n - L + 1) * BLK2 + 36,
                   [[4, 4], [BLK2, NI], [1, 4]]),
                op=ALU.mult)
            nc.vector.tensor_tensor(   # IB: scaled right args x chart[k] FB
                mk(tI, PS, NI * 4, [[2 * NI * 4, 4], [4, NI], [1, 4]]),
                mk(argsI, PS, 4 * NI_MAX, [[0, 4], [4, NI], [1, 4]]),
                mk(chartV, PS, 2 * BLK2 + 52, [[4, 4], [BLK2, NI], [1, 4]]),
                op=ALU.mult)
            nc.vector.tensor_reduce(   # sum over (side*k, arg) -> [PS, 4]
                total36[0:PS, 36:40],
                mk(tI, PS, 0,
                   [[2 * NI * 4, 4], [4, 2 * NI], [1, 4]]),
                axis=AXIS.XY, op=ALU.add)
            nc.vector.tensor_tensor(total36[0:PS, 0:4], total36[0:PS, 0:4],
                                    total36[0:PS, 36:40], op=ALU.add)

        # ---- rescale by a power of 2 near the max (log2-exponent bit
        # tricks on DVE; keeps Exp as the only scalar-engine table in CKY)
        mval = scr.tile([128, 8], F32, tag="mval")
        nc.vector.tensor_reduce(mval[0:PS, 0:1], total36[0:PS, 0:36],
                                axis=AXIS.X, op=ALU.max)
        nc.vector.tensor_scalar(
            mval[0:PS, 4:5].bitcast(I32), mval[0:PS, 0:1].bitcast(I32),
            23, None, op0=ALU.logical_shift_right)          # e = biased exp
        nc.vector.tensor_scalar(
            mval[0:PS, 5:6].bitcast(I32), mval[0:PS, 4:5].bitcast(I32),
            -1, 254, op0=ALU.mult, op1=ALU.add)             # 254 - e
        nc.vector.tensor_scalar(
            mval[0:PS, 6:7].bitcast(I32), mval[0:PS, 5:6].bitcast(I32),
            23, None, op0=ALU.logical_shift_left)           # bits of 2^(127-e)
        nc.vector.tensor_copy(mval[0:PS, 7:8], mval[0:PS, 4:5].bitcast(I32))
        nc.vector.tensor_scalar(
            mval[0:PS, 3:4], mval[0:PS, 7:8],
            127.0, LN2, op0=ALU.subtract, op1=ALU.mult)     # ln(2^(e-127))
        nc.vector.tensor_scalar_mul(
            mk(chartV, PS, L * BLK2, [[1, 36]]),
            total36[0:PS, 0:36], mval[0:PS, 6:7])
        nc.vector.tensor_tensor(
            mkS(chartV, PS, L * BLK2 + 68),
            mstar[0:PS, 0:1], mval[0:PS, 3:4], op=ALU.add)
        nc.vector.tensor_tensor(
            mk(chartV, PS, L * BLK2 + 36, [[4, 4], [1, 4]]),
            mk(chartV, PS, L * BLK2 + 20, [[4, 4], [1, 4]]),
            mk(glR, PS, 0, [[36, 4], [1, 4]]), op=ALU.mult)
        nc.vector.tensor_tensor(
            mk(chartV, PS, L * BLK2 + 52, [[4, 4], [1, 4]]),
            mk(chartV, PS, L * BLK2 + 4, [[4, 4], [1, 4]]),
            mk(grR, PS, 0, [[36, 4], [1, 4]]), op=ALU.mult)
        if L < n:   # chartEnd block L at rows (i+L-1)*4+b
            nc.sync.dma_start(
                mk(chartEV, PS, (n - L) * BLK2, [[1, BLK2]],
                   base_part=4 * (L - 1)),
                mk(chartV, PS, L * BLK2, [[1, BLK2]]))

    # =======================================================================
    # Phase 5: root -> nll per sentence
    # =======================================================================
    nc.vector.tensor_tensor(fin[:, 0:4],
                            mk(chartV, 4, n * BLK2, [[1, 4]]),
                            rsRep[:], op=ALU.mult)
    nc.vector.tensor_reduce(fin[:, 4:5], fin[:, 0:4], axis=AXIS.X, op=ALU.add)
    nc.scalar.activation(fin[:, 5:6], fin[:, 4:5], ACTF.Ln)
    nc.vector.scalar_tensor_tensor(
        fin[:, 6:7], fin[:, 5:6], -1.0,
        mkS(chartV, 4, n * BLK2 + 68),
        op0=ALU.mult, op1=ALU.subtract)
    nc.sync.dma_start(d["out"][:], fin[:, 6:7])
    es2.close()
    es.close()


# ============================================================== host wrapper
_PROG_CACHE = {}


def _get_program(cfg: Cfg):
    key = (cfg.n, cfg.v_loc, cfg.n_cores)
    if key not in _PROG_CACHE:
        _PROG_CACHE[key] = build_program(cfg)
    return _PROG_CACHE[key]


def make_inmaps(cfg: Cfg, inputs):
    """Host-side shard/pack of FULL inputs -> per-core DRAM input dicts."""
    x = np.asarray(inputs["x"])
    check_functor_tables(np.asarray(inputs["l_functors"]),
                         np.asarray(inputs["r_functors"]))
    nt_emb = np.asarray(inputs["nt_emb"], np.float32)          # [C, D]
    vocab_W = np.asarray(inputs["vocab_W"], np.float32)        # [D, V]
    vocab_b = np.asarray(inputs["vocab_b"], np.float32)        # [V]

    import ml_dtypes
    bf16 = ml_dtypes.bfloat16

    ntembT = np.zeros((65, CP), np.float32)
    ntembT[0:64, 0:C] = nt_emb.T
    ntembT[64, :] = 1.0
    ntembT = ntembT.astype(bf16)

    mlpW = np.zeros((64, 322), np.float32)
    for j, k in enumerate(("sW1", "r1W1", "r1W2", "r2W1", "r2W2")):
        mlpW[:, j * 64:(j + 1) * 64] = np.asarray(inputs[k], np.float32)
    mlpW[:, 320:322] = np.asarray(inputs["sW2"], np.float32)
    mlpW = mlpW.astype(bf16)

    mlpB = np.zeros((64, 8), np.float32)
    for j, k in enumerate(("sb1", "r1b1", "r1b2", "r2b1", "r2b2")):
        mlpB[:, j] = np.asarray(inputs[k], np.float32)

    ruleWb = np.zeros((36, 144), np.float32)
    ruleWb[:, 0:72] = np.asarray(inputs["rule_W"], np.float32)
    ruleWb[:, 72:144] = np.tile(
        np.asarray(inputs["rule_b"], np.float32)[None, :], (36, 1))

    smallv = np.zeros((1, 16), np.float32)
    smallv[0, 0:2] = np.asarray(inputs["sb2"], np.float32)
    smallv[0, 2:6] = np.asarray(inputs["root_W"], np.float32)[0, 0:4]
    smallv[0, 6:10] = np.asarray(inputs["root_b"], np.float32)[0:4]

    vs = cfg.v_loc
    in_maps = []
    for core in range(cfg.n_cores):
        vocabW = np.zeros((65, cfg.v_pad), np.float32)
        vocabW[64, :] = NEGB
        vocabW[0:64, 0:vs] = vocab_W[:, core * vs:(core + 1) * vs]
        vocabW[64, 0:vs] = vocab_b[core * vs:(core + 1) * vs]
        vocabW = vocabW.astype(bf16)

        words = x[core * BLOC:(core + 1) * BLOC, 0:cfg.n]   # [BLOC, n]
        wid = words.T.reshape(-1)                           # pair = i*4 + b
        wordW = np.zeros((65, cfg.pairs), np.float32)
        wordW[0:64, :] = vocab_W[:, wid]
        wordW[64, :] = vocab_b[wid]
        wordW = wordW.astype(bf16)

        in_maps.append({
            "ntembT": ntembT, "vocabW": vocabW, "wordW": wordW,
            "mlpW": mlpW, "mlpB": mlpB, "ruleWb": ruleWb, "smallv": smallv,
        })
    return in_maps


def kernel(**inputs) -> np.ndarray:
    cfg = Cfg(n=32, v_loc=V // NCORES, n_cores=NCORES)
    nc = _get_program(cfg)
    in_maps = make_inmaps(cfg, inputs)
    res = bass_utils.run_bass_kernel_spmd(
        nc, in_maps, core_ids=list(range(cfg.n_cores)))
    out = np.concatenate([r["out_nll"].reshape(-1) for r in res.results])
    return out.astype(np.float32)


if __name__ == "__main__":
    from reference import setup_inputs, reference
    inputs = {k: np.asarray(v) for k, v in setup_inputs().items()}
    got = kernel(**inputs)
    exp = np.asarray(reference(**inputs))
    rel = np.max(np.abs(got - exp) / np.maximum(np.abs(exp), 1e-6))
    print("expected:", exp[:8])
    print("got     :", got[:8])
    print("Relative error:", rel)

